# revision 41
# baseline (speedup 1.0000x reference)
"""Trainium2 Bass kernel for nn_CLIP_69458211111620 (v3: folded head).

Data-parallel over batch B=128 across 8 NeuronCores (16 batches/core).
Single fused pass per batch (no DRAM staging), software-pipelined 4 deep:
  P1(i): emb matmuls + LN + transpose -> S
  P2(i-1): RT/ab/sc/V/scoresT matmuls, exp (pre-transposed softmax), fused, LN
  P3(i-2): fNT transpose, pooling MLP hT
  P4(i-3): pool softmax + pooled row (batch-partition layout)
then fc head: Wf2@Wf3 is host-folded to a single vector (no nonlinearity
between them), so the head is pooled @ Wf1' -> lrelu -> @ W23 with
LN_f's affine folded into Wf1'/bf1' as well.

Precision: bf16 matmuls, f32r where critical (fp8 on the attention chain
exceeds the 2e-2 budget per earlier ablations).
"""
import sys

sys.path.insert(0, "/opt/trn_rl_repo")

import numpy as np
import ml_dtypes

NCORES = 8
NB = 16          # batches per core
T, C, D, DF, H = 1024, 512, 1024, 2048, 64
ISD = 1.0 / 32.0  # 1/sqrt(D)
EPS = 1e-5

# ---- precision config (validated by sim.py ablations) -----------------
# fp8 e4m3 on any attention-chain operand exceeds the 2e-2 budget (the
# softmax sharply amplifies quantization noise); all-bf16 sims at 6.6e-3.
EMB_FP8 = False   # x, wemb e4m3 (DoubleRow)
ATTN_FP8 = False  # S, M, wqb e4m3 -> RT/ab/sc DoubleRow
V_FP8 = False     # wv e4m3 (uses e4m3 S) -> V matmul DoubleRow
ABSC_DT = "bf16"  # scores matmul operand dtype ("bf16" | "f32r")
HEAD_DT = "bf16"  # fc head weight dtype ("bf16" | "f32r")

S_EMB = 64.0 if EMB_FP8 else 1.0
S_M = 16.0 if ATTN_FP8 else 1.0
S_WQB = 16.0 if ATTN_FP8 else 1.0
S_WV = 32.0 if V_FP8 else 1.0

E4NP = ml_dtypes.float8_e4m3
BFNP = ml_dtypes.bfloat16


def _round_f32r(x):
    u = np.ascontiguousarray(x, dtype=np.float32).view(np.uint32).copy()
    lsb = (u >> np.uint32(12)) & np.uint32(1)
    u += np.uint32(0x7FF) + lsb
    u &= np.uint32(0xFFFFF000)
    return u.view(np.float32)


def _chunk_major(v, nchunk):
    return np.ascontiguousarray(
        np.asarray(v, dtype=np.float32).reshape(nchunk, 128).T
    )


def _sbuf_layout(w, nk):
    """[nk*128, F] -> [128, nk, F] partition-major image."""
    w = np.asarray(w, dtype=np.float32)
    f = w.shape[1]
    return np.ascontiguousarray(
        w.reshape(nk, 128, f).transpose(1, 0, 2)
    )


def _hilo_rows(v):
    """[N] -> [2, N] bf16 (hi, lo) rows."""
    v = np.asarray(v, np.float32)
    hi = v.astype(BFNP)
    lo = (v - hi.astype(np.float32)).astype(BFNP)
    return np.ascontiguousarray(np.stack([hi, lo], axis=0))


def _cast(x, dt):
    if dt == "e4m3":
        return np.clip(x, -240.0, 240.0).astype(E4NP)
    if dt == "bf16":
        return np.asarray(x, np.float32).astype(BFNP)
    return _round_f32r(x)


def _build(age_scale_f, b23_f, bemb_nz, bv_nz, s_affine_triv=True,
           sim_acts=False):
    import concourse.tile as tile
    import concourse.bass as bass
    from concourse import bacc, mybir

    F32 = mybir.dt.float32
    F32R = mybir.dt.float32r
    BF16 = mybir.dt.bfloat16
    E4M3 = mybir.dt.float8e4
    AF = mybir.ActivationFunctionType
    ALU = mybir.AluOpType
    AX = mybir.AxisListType
    PM = mybir.MatmulPerfMode
    ts = bass.ts
    AF_LRELU = AF.Relu if sim_acts else AF.Lrelu

    I32 = mybir.dt.int32
    RSQRT_MAGIC = 0x5F3759DF

    SDT = E4M3 if (ATTN_FP8 or V_FP8) else BF16
    XDT = E4M3 if EMB_FP8 else BF16
    WEDT = E4M3 if EMB_FP8 else BF16
    MDT = E4M3 if ATTN_FP8 else BF16
    WVDT = E4M3 if V_FP8 else BF16
    ABDT = F32R if ABSC_DT == "f32r" else BF16
    HDT = F32R if HEAD_DT == "f32r" else BF16

    def kch(n, fp8):
        """Chunk iteration: DoubleRow pairs if fp8 else single chunks."""
        if fp8:
            return [
                (slice(2 * p, 2 * p + 2), p == 0, p == n // 2 - 1, PM.DoubleRow)
                for p in range(n // 2)
            ]
        return [(slice(k, k + 1), k == 0, k == n - 1, None) for k in range(n)]

    nc = bacc.Bacc("TRN2", target_bir_lowering=False, debug=False)

    def inp(name, shape, dt):
        return nc.dram_tensor(name, shape, dt, kind="ExternalInput").ap()

    X = inp("x", (NB, 128, 8, C), XDT)
    WEMB = inp("wemb", (128, 8, D), WEDT)
    MQK = inp("m_mat", (128, 8, D), MDT)
    WQB = inp("wqb", (128, 8, C), MDT)
    WV = inp("wv", (128, 8, D), WVDT)
    WP1G = inp("wp1g", (128, 8, 128), BF16)
    WP2 = inp("wp2", (H, 1), BF16)
    WF1 = inp("wf1", (32, 128, 512), HDT)
    W23BC = inp("w23bc", (NB, DF), F32)
    GS = inp("gs_c", (128, 8), F32)
    BS = inp("bs_c", (128, 8), F32)
    BP1E = inp("bp1e", (H, 1), F32)
    BF1R = inp("bf1r", (2, DF), BF16)
    IDENT = inp("ident", (128, 128), BF16)
    BEMB = inp("bemb_row", (1, D), F32R) if bemb_nz else None
    BVR = inp("bv_row", (1, D), F32R) if bv_nz else None
    RUL = nc.dram_tensor("rul", (NB, 1), F32, kind="ExternalOutput").ap()

    with tile.TileContext(nc) as tc:
        # ---- long-lived tiles ----------------------------------------
        glob = tc.alloc_tile_pool(name="glob", bufs=1)
        id_sb = glob.tile([128, 128], BF16, name="id_sb")
        magic_t = glob.tile([128, 4], I32, name="magic_t")
        ages_t = glob.tile([128, 1], F32, name="ages_t")
        ones_c = glob.tile([128, 1], BF16, name="ones_c")
        idf1 = glob.tile([1, 1], F32, name="idf1")
        pool16 = glob.tile([NB, D], BF16, name="pool16")
        ones1 = glob.tile([2, NB], BF16, name="ones1")
        gs_sb = glob.tile([128, 8], F32, name="gs_sb")
        bs_sb = glob.tile([128, 8], F32, name="bs_sb")
        nc.sync.dma_start(id_sb[:], IDENT[:])
        nc.sync.dma_start(gs_sb[:], GS[:])
        nc.sync.dma_start(bs_sb[:], BS[:])
        nc.gpsimd.memset(magic_t[:], RSQRT_MAGIC)
        nc.gpsimd.memset(ages_t[:], age_scale_f)
        nc.gpsimd.memset(ones_c[:], 1.0)
        nc.gpsimd.memset(idf1[:], 1.0)
        nc.gpsimd.memset(ones1[:], 1.0)
        ones_r = None
        if bemb_nz or bv_nz:
            ones_r = glob.tile([1, 128], F32R, name="ones_r")
            nc.gpsimd.memset(ones_r[:], 1.0)
        bemb_sb = None
        if bemb_nz:
            bemb_sb = glob.tile([1, D], F32R, name="bemb_sb")
            nc.sync.dma_start(bemb_sb[:], BEMB[:])
        bv_sb = None
        if bv_nz:
            bv_sb = glob.tile([1, D], F32R, name="bv_sb")
            nc.sync.dma_start(bv_sb[:], BVR[:])

        # ---- weights --------------------------------------------------
        wts = tc.alloc_tile_pool(name="wts", bufs=1)
        wemb_sb = wts.tile([128, 8, D], WEDT, name="wemb_sb")
        m_sb = wts.tile([128, 8, D], MDT, name="m_sb")
        wqb_sb = wts.tile([128, 8, C], MDT, name="wqb_sb")
        wv_sb = wts.tile([128, 8, D], WVDT, name="wv_sb")
        wp1_sb = wts.tile([128, 8, 128], BF16, name="wp1_sb")
        wp2_sb = wts.tile([H, 1], BF16, name="wp2_sb")
        bp1_sb = wts.tile([H, 1], F32, name="bp1_sb")
        bf1r_sb = wts.tile([2, DF], BF16, name="bf1r_sb")
        w23bc_sb = wts.tile([NB, DF], F32, name="w23bc_sb")
        pooledT = wts.tile([128, 8, NB], BF16, name="pooledT")
        weight_dmas = [
            (m_sb, MQK), (wqb_sb, WQB), (wv_sb, WV), (wp1_sb, WP1G),
            (wp2_sb, WP2), (bp1_sb, BP1E), (bf1r_sb, BF1R), (w23bc_sb, W23BC),
        ]

        def emit_rsqrt(pool, v_ap, w, tagp, eps, iters=1):
            """[128,w] -> 1/sqrt(v + eps) elementwise on DVE (Quake+Newton)."""
            ve = pool.tile([128, w], F32, name=f"{tagp}ve", tag=f"{tagp}ve")
            nc.vector.tensor_scalar(ve[:], v_ap, eps, None, op0=ALU.add)
            y = pool.tile([128, w], F32, name=f"{tagp}y0", tag=f"{tagp}y0")
            nc.vector.tensor_scalar(
                y.bitcast(I32)[:], ve.bitcast(I32)[:], 1, None,
                op0=ALU.logical_shift_right,
            )
            nc.vector.scalar_tensor_tensor(
                y.bitcast(I32)[:], y.bitcast(I32)[:], -1, magic_t[:, 0:w],
                op0=ALU.mult, op1=ALU.add,
            )
            for it in range(iters):
                a = pool.tile([128, w], F32, name=f"{tagp}a{it}", tag=f"{tagp}a{it}")
                nc.vector.tensor_tensor(a[:], y[:], y[:], op=ALU.mult)
                nc.vector.tensor_tensor(a[:], a[:], ve[:], op=ALU.mult)
                nc.vector.tensor_scalar(
                    a[:], a[:], -0.5, 1.5, op0=ALU.mult, op1=ALU.add
                )
                nc.vector.tensor_tensor(y[:], y[:], a[:], op=ALU.mult)
            return y

        # ---- pipelined main loop -------------------------------------
        with (
            tc.tile_pool(name="px", bufs=2) as px,
            tc.tile_pool(name="pw3", bufs=16) as pw3,
            tc.tile_pool(name="psen", bufs=1) as psen,
            tc.tile_pool(name="pS", bufs=2) as pS,
            tc.tile_pool(name="pmid", bufs=1) as pmid,
            tc.tile_pool(name="pfn", bufs=3) as pfn,
            tc.tile_pool(name="psc1", bufs=2) as psc1,
            tc.tile_pool(name="psc2", bufs=2) as psc2,
            tc.tile_pool(name="pgel", bufs=1) as pgel,
            tc.tile_pool(name="ps_emb", bufs=3, space="PSUM") as ps_emb,
            tc.tile_pool(name="ps_main", bufs=3, space="PSUM") as ps_main,
            tc.tile_pool(name="ps_small", bufs=2, space="PSUM") as ps_small,
        ):
            st = [dict() for _ in range(NB)]

            def p1_emb(i):
                s = st[i]
                if i == 0:
                    # split first-batch DMAs so ck0/dh0 compute starts early
                    xb = px.tile([128, 8, C], XDT, name="xb", tag="xb")
                    nc.sync.dma_start(xb[:, :, 0:128], X[0][:, :, 0:128])
                    nc.sync.dma_start(
                        wemb_sb[:, :, 0:512], WEMB[:, :, 0:512]
                    )
                    nc.sync.dma_start(xb[:, :, 128:512], X[0][:, :, 128:512])
                    nc.sync.dma_start(
                        wemb_sb[:, :, 512:1024], WEMB[:, :, 512:1024]
                    )
                    for w_t, w_d in weight_dmas:
                        nc.sync.dma_start(w_t[:], w_d[:])
                    s["xb"] = xb
                xb = s.pop("xb")
                if i + 1 < NB:
                    xb2 = px.tile([128, 8, C], XDT, name="xb2", tag="xb")
                    nc.sync.dma_start(xb2[:], X[i + 1])
                    st[i + 1]["xb"] = xb2
                sen_n = psen.tile([128, 4, D], BF16, name="sen_n", tag="sen")
                for ck in range(4):
                    bn6 = psc1.tile([128, 2, 6], F32, name="bn6", tag="st6")
                    ph2 = []
                    for dh in range(2):
                        ps_s = ps_emb.tile([128, 512], F32, name="ps_s", tag="ps_s")
                        for sl, sta, stp, pm in kch(8, EMB_FP8):
                            nc.tensor.matmul(
                                ps_s[:],
                                xb[:, sl, ts(ck, 128)],
                                wemb_sb[:, sl, dh * 512:(dh + 1) * 512],
                                start=sta,
                                stop=(stp and not bemb_nz),
                                perf_mode=pm,
                            )
                        if bemb_nz:
                            nc.tensor.matmul(
                                ps_s[:],
                                ones_r[0:1, :],
                                bemb_sb[0:1, dh * 512:(dh + 1) * 512],
                                start=False, stop=True,
                            )
                        nc.vector.bn_stats(bn6[:, dh, :], ps_s[:])
                        ph2.append(ps_s)
                    bnag = psc1.tile([128, 2], F32, name="bnag", tag="bnag")
                    nc.vector.bn_aggr(bnag[:], bn6[:])
                    i_t = emit_rsqrt(
                        psc1, bnag[:, 1:2], 1, "l1", EPS * S_EMB * S_EMB
                    )
                    negmi = psc1.tile([128, 1], F32, name="negmi", tag="negmi")
                    nc.vector.scalar_tensor_tensor(
                        negmi[:], bnag[:, 0:1], -1.0, i_t[:],
                        op0=ALU.mult, op1=ALU.mult,
                    )
                    for dh in range(2):
                        nc.scalar.activation(
                            sen_n[:, ck, dh * 512:(dh + 1) * 512],
                            ph2[dh][:], AF.Identity,
                            bias=negmi[:], scale=i_t[:],
                        )
                if s_affine_triv:
                    # g_s==1, b_s==0: S_t is a pure transpose -> DMA xbar
                    S_t = pS.tile([128, 8, C], SDT, name="S_t", tag="S")
                    for ck in range(4):
                        nc.sync.dma_start_transpose(
                            S_t[:, 0:8, ts(ck, 128)], sen_n[:, ck, :]
                        )
                    s["S"] = S_t
                else:
                    s["sen_n"] = sen_n

            def p1_tr(i):
                # fallback path: PE transpose + affine (general g_s/b_s)
                s = st[i]
                if "sen_n" not in s:
                    return
                sen_n = s.pop("sen_n")
                S_t = pS.tile([128, 8, C], SDT, name="S_t", tag="S")
                for dk in range(8):
                    ps_t = ps_small.tile([128, 512], BF16, name="ps_t", tag="sm")
                    for ck in range(4):
                        nc.tensor.transpose(
                            ps_t[:, ts(ck, 128)], sen_n[:, ck, ts(dk, 128)],
                            id_sb[:],
                        )
                    nc.scalar.activation(
                        S_t[:, dk, :], ps_t[:], AF.Identity,
                        bias=bs_sb[:, dk:dk + 1], scale=gs_sb[:, dk:dk + 1],
                    )
                s["S"] = S_t

            def p2_big(i):
                s = st[i]
                S_t = s.pop("S")
                # RT = (S M)^T  [e(8), n=C]
                RT = pmid.tile([128, 8, C], MDT, name="RT", tag="RT")
                for ec in range(8):
                    ptr = ps_main.tile([128, C], F32, name="ptr", tag="pm")
                    for sl, sta, stp, pm in kch(8, ATTN_FP8):
                        nc.tensor.matmul(
                            ptr[:], m_sb[:, sl, ts(ec, 128)], S_t[:, sl, :],
                            start=sta, stop=stp, perf_mode=pm,
                        )
                    nc.scalar.activation(RT[:, ec, :], ptr[:], AF.Copy)
                # ab = S Wqb * isd/s_wqb  [n(4), m=C]
                ab = pmid.tile([128, 4, C], ABDT, name="ab", tag="ab")
                for nk in range(4):
                    pa = ps_main.tile([128, C], F32, name="pa", tag="pm")
                    for sl, sta, stp, pm in kch(8, ATTN_FP8):
                        nc.tensor.matmul(
                            pa[:], S_t[:, sl, ts(nk, 128)], wqb_sb[:, sl, :],
                            start=sta, stop=stp, perf_mode=pm,
                        )
                    nc.scalar.activation(
                        ab[:, nk, :], pa[:], AF.Copy, scale=float(ISD / S_WQB)
                    )
                # sc = R S^T * isd/s_M + age  [n(4), m=C]
                sc = pmid.tile([128, 4, C], ABDT, name="sc", tag="sc")
                for nk in range(4):
                    pa = ps_main.tile([128, C], F32, name="pa2", tag="pm")
                    for sl, sta, stp, pm in kch(8, ATTN_FP8):
                        nc.tensor.matmul(
                            pa[:], RT[:, sl, ts(nk, 128)], S_t[:, sl, :],
                            start=sta, stop=stp, perf_mode=pm,
                        )
                    nc.scalar.activation(
                        sc[:, nk, :], pa[:], AF.Identity,
                        bias=ages_t[:], scale=float(ISD / S_M),
                    )
                # V = S Wv  [m(4), D]
                V = pmid.tile([128, 4, D], BF16, name="V", tag="V")
                for mk in range(4):
                    for dh in range(2):
                        pv = ps_main.tile([128, 512], F32, name="pv", tag="pm")
                        for sl, sta, stp, pm in kch(8, V_FP8):
                            nc.tensor.matmul(
                                pv[:],
                                S_t[:, sl, ts(mk, 128)],
                                wv_sb[:, sl, dh * 512:(dh + 1) * 512],
                                start=sta,
                                stop=(stp and not bv_nz),
                                perf_mode=pm,
                            )
                        if bv_nz:
                            nc.tensor.matmul(
                                pv[:],
                                ones_r[0:1, :],
                                bv_sb[0:1, dh * 512:(dh + 1) * 512],
                                start=False, stop=True,
                            )
                        nc.vector.tensor_copy(
                            V[:, mk, dh * 512:(dh + 1) * 512], pv[:]
                        )
                s["V"] = V
                # scoresT[k,n] = sum_j sc[j,k] ab[j,n]; exp -> expT (bf16)
                expT = pmid.tile([128, 4, C], BF16, name="expT", tag="expT")
                for kk in range(4):
                    psc = ps_main.tile([128, C], F32, name="psc", tag="pm")
                    for jk in range(4):
                        nc.tensor.matmul(
                            psc[:], sc[:, jk, ts(kk, 128)], ab[:, jk, :],
                            start=(jk == 0), stop=(jk == 3),
                        )
                    nc.scalar.activation(expT[:, kk, :], psc[:], AF.Exp)
                s["expT"] = expT

            def p2_fused(i):
                s = st[i]
                V = s.pop("V")
                expT = s.pop("expT")
                # row sums of exp (over k) as columns per nk + reciprocal
                pssum = ps_small.tile([128, 4], F32, name="pssum", tag="sm")
                for nk in range(4):
                    for kk in range(4):
                        nc.tensor.matmul(
                            pssum[:, nk:nk + 1],
                            expT[:, kk, ts(nk, 128)],
                            ones_c[:],
                            start=(kk == 0), stop=(kk == 3),
                        )
                recips = psc2.tile([128, 4], F32, name="recips", tag="rec")
                nc.vector.reciprocal(recips[:], pssum[:])
                # fused = softmax @ V * isd (LN folded)
                fN = pfn.tile([128, 4, D], BF16, name="fN", tag="fN")
                bn6f = psc2.tile([128, 2, 6], F32, name="bn6f", tag="bn6f")
                bnagf = psc2.tile([128, 2], F32, name="bnagf", tag="bnagf")
                for nk in range(4):
                    pfs = []
                    for dh in range(2):
                        pf = ps_main.tile([128, 512], F32, name="pf", tag="pm")
                        for kk in range(4):
                            nc.tensor.matmul(
                                pf[:],
                                expT[:, kk, ts(nk, 128)],
                                V[:, kk, dh * 512:(dh + 1) * 512],
                                start=(kk == 0), stop=(kk == 3),
                            )
                        nc.vector.bn_stats(bn6f[:, dh, :], pf[:])
                        pfs.append(pf)
                    nc.vector.bn_aggr(bnagf[:], bn6f[:])
                    s_t = psc2.tile([128, 1], F32, name="s_t", tag="s_t")
                    nc.vector.tensor_scalar(
                        s_t[:], recips[:, nk:nk + 1], float(ISD / S_WV), None,
                        op0=ALU.mult,
                    )
                    s2_t = psc2.tile([128, 1], F32, name="s2_t", tag="s2_t")
                    nc.vector.tensor_tensor(s2_t[:], s_t[:], s_t[:], op=ALU.mult)
                    vs_t = psc2.tile([128, 1], F32, name="vs_t", tag="vs_t")
                    nc.vector.scalar_tensor_tensor(
                        vs_t[:], bnagf[:, 1:2], 1.0, s2_t[:],
                        op0=ALU.mult, op1=ALU.mult,
                    )
                    i2_t = emit_rsqrt(psc2, vs_t[:], 1, "l2", EPS)
                    se_t = psc2.tile([128, 1], F32, name="se_t", tag="se_t")
                    nc.vector.tensor_tensor(se_t[:], s_t[:], i2_t[:], op=ALU.mult)
                    be_t = psc2.tile([128, 1], F32, name="be_t", tag="be_t")
                    nc.vector.scalar_tensor_tensor(
                        be_t[:], bnagf[:, 0:1], -1.0, se_t[:],
                        op0=ALU.mult, op1=ALU.mult,
                    )
                    for dh in range(2):
                        nc.scalar.activation(
                            fN[:, nk, dh * 512:(dh + 1) * 512], pfs[dh][:],
                            AF.Identity, bias=be_t[:], scale=se_t[:],
                        )
                # transpose fN -> fNT on the DMA xbar (consumed by p3 next
                # iteration, so the DMA latency is fully hidden)
                fNT = pmid.tile([128, 8, C], BF16, name="fNT", tag="fNT")
                for nk in range(4):
                    nc.sync.dma_start_transpose(
                        fNT[:, 0:8, ts(nk, 128)], fN[:, nk, :]
                    )
                s["fNT"] = fNT
                s["fN"] = fN

            def p3(i):
                s = st[i]
                fNT = s.pop("fNT")
                ph = ps_main.tile([128, C], F32, name="ph", tag="pm")
                for kc in range(8):
                    nc.tensor.matmul(
                        ph[:], wp1_sb[:, kc, :], fNT[:, kc, :],
                        start=(kc == 0), stop=(kc == 7),
                    )
                # gelu (tanh formula; Square/Tanh share the Exp table set)
                gx = pgel.tile([H, C], F32, name="gx", tag="gx")
                nc.scalar.activation(gx[:], ph[0:H, :], AF.Identity, bias=bp1_sb[:])
                g2 = pgel.tile([H, C], F32, name="g2", tag="g2")
                nc.scalar.activation(g2[:], gx[:], AF.Square)
                nc.vector.tensor_scalar(
                    g2[:], g2[:], 0.044715 * 0.7978845608028654,
                    0.7978845608028654, op0=ALU.mult, op1=ALU.add,
                )
                nc.vector.tensor_tensor(g2[:], g2[:], gx[:], op=ALU.mult)
                nc.scalar.activation(g2[:], g2[:], AF.Tanh)
                nc.vector.tensor_scalar(g2[:], g2[:], 1.0, None, op0=ALU.add)
                hT = pgel.tile([H, C], BF16, name="hT", tag="hT")
                nc.vector.scalar_tensor_tensor(
                    hT[:], g2[:], 0.5, gx[:], op0=ALU.mult, op1=ALU.mult,
                )
                s["hT"] = hT

            def p4a(i):
                s = st[i]
                hT = s.pop("hT")
                pps = ps_main.tile([1, C], F32, name="pps", tag="pm")
                nc.tensor.matmul(pps[:], wp2_sb[:], hT[:], start=True, stop=True)
                pnm = psc2.tile([1, 1], F32, name="pnm", tag="pnm")
                nc.vector.tensor_reduce(
                    pnm[:], pps[:], axis=AX.X, op=ALU.max, negate=True
                )
                pw = pgel.tile([1, C], BF16, name="pw", tag="row")
                pse = psc2.tile([1, 1], F32, name="pse", tag="pse")
                nc.scalar.activation(
                    pw[:], pps[:], AF.Exp, bias=pnm[:], accum_out=pse[:]
                )
                prc = psc2.tile([1, 1], F32, name="prc", tag="prc")
                nc.vector.reciprocal(prc[:], pse[:])
                pwn = pgel.tile([1, C], F32, name="pwn", tag="rown")
                nc.vector.tensor_scalar(
                    pwn[:], pw[:], prc[0:1, 0:1], None, op0=ALU.mult
                )
                s["pwn"] = pwn

            def p4b(i):
                s = st[i]
                pwn = s.pop("pwn")
                fN = s.pop("fN")
                ppw = ps_small.tile([128, 4], F32, name="ppw", tag="sm")
                for nk in range(4):
                    nc.tensor.transpose(
                        ppw[:, nk:nk + 1], pwn[0:1, ts(nk, 128)], idf1[:]
                    )
                pwc = pgel.tile([128, 4], BF16, name="pwc", tag="pwc")
                nc.scalar.activation(pwc[:], ppw[:], AF.Copy)
                # pooled row (LN_f affine deferred into Wf1'/bf1' host folds)
                prow_sb = pgel.tile([1, D], BF16, name="prow", tag="prow")
                for dh in range(2):
                    prow = ps_small.tile([1, 512], F32, name="prow_ps", tag="sm")
                    for nk in range(4):
                        nc.tensor.matmul(
                            prow[:], pwc[:, nk:nk + 1],
                            fN[:, nk, dh * 512:(dh + 1) * 512],
                            start=(nk == 0), stop=(nk == 3),
                        )
                    nc.vector.tensor_copy(
                        prow_sb[0:1, dh * 512:(dh + 1) * 512], prow[:]
                    )
                nc.sync.dma_start(pool16[i:i + 1, :], prow_sb[0:1, :])

            for i in range(NB + 3):
                if i < NB:
                    p1_emb(i)
                if 1 <= i <= NB:
                    p2_big(i - 1)
                if 3 <= i:
                    p4a(i - 3)
                if 1 <= i <= NB:
                    p2_fused(i - 1)
                if 3 <= i:
                    p4b(i - 3)
                if i < NB:
                    p1_tr(i)
                if 2 <= i <= NB + 1:
                    p3(i - 2)

            # =================== fc head ==============================
            # Wf2 @ Wf3 folded host-side (no nonlinearity between them):
            # rul = |lrelu(pooled @ Wf1' + bf1') @ W23 + b23|
            wts1 = []
            for t in range(32):
                wt = pw3.tile([128, 512], HDT, name="wt1", tag="w3")
                nc.sync.dma_start(wt[:], WF1[t])
                wts1.append(wt)
            # pooledT [d-part, batch] from pool16 [batch, d]
            for dk in range(8):
                ptp = ps_small.tile([128, NB], BF16, name="ptp", tag="sm")
                nc.tensor.transpose(
                    ptp[:], pool16[0:NB, ts(dk, 128)], id_sb[0:NB, 0:NB]
                )
                nc.scalar.activation(pooledT[:, dk, :], ptp[:], AF.Copy)
            # h1 = lrelu(pooled @ Wf1' + bf1')  [batch-part 16, DF]
            h1 = pgel.tile([NB, DF], BF16, name="h1", tag="h1")
            for fg in range(4):
                hg = ps_main.tile([NB, 512], F32, name="hg", tag="pm")
                for kd in range(8):
                    nc.tensor.matmul(
                        hg[:], pooledT[:, kd, :], wts1[fg * 8 + kd][:],
                        start=(kd == 0), stop=False,
                    )
                nc.tensor.matmul(
                    hg[:], ones1[0:2, :], bf1r_sb[0:2, ts(fg, 512)],
                    start=False, stop=True,
                )
                nc.scalar.activation(
                    h1[0:NB, ts(fg, 512)], hg[:], AF_LRELU, alpha=0.01
                )
            # rul = |h1 . W23 + b23| as a per-partition DVE dot (batch-part)
            rsum4 = psc2.tile([NB, 4], F32, name="rsum4", tag="rsum4")
            for fg in range(4):
                hm = pgel.tile([NB, 512], F32, name="hm", tag="hm")
                nc.vector.tensor_tensor(
                    hm[:], h1[0:NB, ts(fg, 512)], w23bc_sb[0:NB, ts(fg, 512)],
                    op=ALU.mult,
                )
                nc.vector.tensor_reduce(
                    rsum4[:, fg:fg + 1], hm[:], axis=AX.X, op=ALU.add
                )
            rsum = psc2.tile([NB, 1], F32, name="rsum", tag="rsum")
            nc.vector.tensor_reduce(rsum[:], rsum4[:], axis=AX.X, op=ALU.add)
            bf3_t = psc2.tile([NB, 1], F32, name="bf3_t", tag="bf3")
            nc.gpsimd.memset(bf3_t[:], b23_f)
            rul_sb = psc2.tile([NB, 1], F32, name="rul_sb", tag="rul")
            nc.scalar.activation(rul_sb[:], rsum[:], AF.Abs, bias=bf3_t[:])
            nc.sync.dma_start(RUL[:], rul_sb[:])

        wts.release()
        glob.release()

    nc.compile()
    return nc


def _prep_in_maps(inputs):
    f32 = np.float32
    x_enc = np.asarray(inputs["x_enc"], f32)
    W_emb = np.asarray(inputs["W_emb"], f32)
    b_emb = np.asarray(inputs["b_emb"], f32)
    g_s = np.asarray(inputs["g_s"], f32)
    b_s = np.asarray(inputs["b_s"], f32)
    basis = np.asarray(inputs["basis"], np.float64)
    Wq = np.asarray(inputs["Wq"], np.float64)
    bq = np.asarray(inputs["bq"], f32)
    Wk = np.asarray(inputs["Wk"], np.float64)
    bk = np.asarray(inputs["bk"], f32)
    Wv = np.asarray(inputs["Wv"], f32)
    bv = np.asarray(inputs["bv"], f32)
    g_f = np.asarray(inputs["g_f"], f32)
    b_f = np.asarray(inputs["b_f"], f32)
    Wp1 = np.asarray(inputs["Wp1"], f32)
    bp1 = np.asarray(inputs["bp1"], f32)
    Wp2 = np.asarray(inputs["Wp2"], f32)
    Wf1 = np.asarray(inputs["Wf1"], np.float64)
    bf1 = np.asarray(inputs["bf1"], np.float64)
    Wf2 = np.asarray(inputs["Wf2"], np.float64)
    bf2 = np.asarray(inputs["bf2"], np.float64)
    Wf3 = np.asarray(inputs["Wf3"], np.float64)
    b_f64 = np.asarray(inputs["b_f"], np.float64)
    g_f64 = np.asarray(inputs["g_f"], np.float64)

    assert not (np.any(bq) or np.any(bk)), "folded QK path requires bq=bk=0"

    M = (Wq @ Wk.T).astype(f32)
    Wqb = (Wq @ basis.T).astype(f32)

    wp1g = np.zeros((D, 128), f32)
    wp1g[:, :H] = g_f[:, None] * Wp1
    bp1e = (b_f @ Wp1 + bp1).reshape(H, 1).astype(f32)

    # head folds: LN_f affine into Wf1/bf1; Wf2@Wf3 into one vector
    Wf1g = (g_f64[:, None] * Wf1).astype(f32)
    bf1g = (bf1 + b_f64 @ Wf1).astype(f32)
    W23 = (Wf2 @ Wf3[:, 0]).astype(f32)
    b23 = float(bf2 @ Wf3[:, 0])

    e_dt = "e4m3" if EMB_FP8 else "bf16"
    m_dt = "e4m3" if ATTN_FP8 else "bf16"
    v_dt = "e4m3" if V_FP8 else "bf16"
    h_dt = "bf16" if HEAD_DT == "bf16" else "f32r"

    common = {
        "wemb": _cast(_sbuf_layout(W_emb * S_EMB, 8), e_dt),
        "m_mat": _cast(_sbuf_layout(M * S_M, 8), m_dt),
        "wqb": _cast(_sbuf_layout(Wqb * S_WQB, 8), m_dt),
        "wv": _cast(_sbuf_layout(Wv * S_WV, 8), v_dt),
        "wp1g": _cast(_sbuf_layout(wp1g, 8), "bf16"),
        "wp2": Wp2.astype(BFNP),
        "wf1": _cast(
            Wf1g.reshape(8, 128, 4, 512).transpose(2, 0, 1, 3).reshape(32, 128, 512),
            h_dt,
        ),
        "w23bc": np.ascontiguousarray(np.tile(W23.reshape(1, DF), (NB, 1))),
        "gs_c": _chunk_major(g_s, 8),
        "bs_c": _chunk_major(b_s, 8),
        "bp1e": bp1e,
        "bf1r": _hilo_rows(bf1g),
        "ident": np.eye(128).astype(BFNP),
    }
    bemb_nz = bool(np.any(b_emb))
    bv_nz = bool(np.any(bv))
    if bemb_nz:
        common["bemb_row"] = _round_f32r(b_emb.reshape(1, D) * S_EMB)
    if bv_nz:
        common["bv_row"] = _round_f32r(bv.reshape(1, D) * S_WV)

    in_maps = []
    for c in range(NCORES):
        m = dict(common)
        xs = x_enc[c * NB:(c + 1) * NB]
        m["x"] = _cast(
            xs.reshape(NB, 8, 128, C).transpose(0, 2, 1, 3), e_dt
        )
        in_maps.append(m)

    age_scale_f = float(np.asarray(inputs["age_scale"], f32))
    bf3_f = float(np.asarray(inputs["bf3"], np.float64).reshape(-1)[0])
    b23_f = b23 + bf3_f
    s_triv = bool(np.allclose(g_s, 1.0) and not np.any(b_s))
    return in_maps, age_scale_f, b23_f, bemb_nz, bv_nz, s_triv


_NC_CACHE = {}


def build_program(inputs, sim_acts=False):
    in_maps, age_scale_f, b23_f, bemb_nz, bv_nz, s_triv = _prep_in_maps(inputs)
    key = (age_scale_f, b23_f, bemb_nz, bv_nz, s_triv, sim_acts)
    if key not in _NC_CACHE:
        _NC_CACHE[key] = _build(
            age_scale_f, b23_f, bemb_nz, bv_nz, s_triv, sim_acts
        )
    return _NC_CACHE[key], in_maps


def kernel(**inputs):
    from concourse.bass_utils import run_bass_kernel_spmd

    nc, in_maps = build_program(inputs)
    res = run_bass_kernel_spmd(nc, in_maps, core_ids=list(range(NCORES)))
    out = np.concatenate(
        [res.results[c]["rul"] for c in range(NCORES)], axis=0
    ).astype(np.float32)
    return out



# revision 46
# speedup vs baseline: 1.0472x; 1.0472x over previous
"""Trainium2 Bass kernel for nn_CLIP_69458211111620 (v3: folded head).

Data-parallel over batch B=128 across 8 NeuronCores (16 batches/core).
Single fused pass per batch (no DRAM staging), software-pipelined 4 deep:
  P1(i): emb matmuls + LN + transpose -> S
  P2(i-1): RT/ab/sc/V/scoresT matmuls, exp (pre-transposed softmax), fused, LN
  P3(i-2): fNT transpose, pooling MLP hT
  P4(i-3): pool softmax + pooled row (batch-partition layout)
then fc head: Wf2@Wf3 is host-folded to a single vector (no nonlinearity
between them), so the head is pooled @ Wf1' -> lrelu -> @ W23 with
LN_f's affine folded into Wf1'/bf1' as well.

Precision: bf16 matmuls, f32r where critical (fp8 on the attention chain
exceeds the 2e-2 budget per earlier ablations).
"""
import sys

sys.path.insert(0, "/opt/trn_rl_repo")

import numpy as np
import ml_dtypes

NCORES = 8
NB = 16          # batches per core
T, C, D, DF, H = 1024, 512, 1024, 2048, 64
ISD = 1.0 / 32.0  # 1/sqrt(D)
EPS = 1e-5

# ---- precision config (validated by sim.py ablations) -----------------
# fp8 e4m3 on any attention-chain operand exceeds the 2e-2 budget (the
# softmax sharply amplifies quantization noise); all-bf16 sims at 6.6e-3.
EMB_FP8 = False   # x, wemb e4m3 (DoubleRow)
ATTN_FP8 = False  # S, M, wqb e4m3 -> RT/ab/sc DoubleRow
V_FP8 = False     # wv e4m3 (uses e4m3 S) -> V matmul DoubleRow
ABSC_DT = "bf16"  # scores matmul operand dtype ("bf16" | "f32r")
HEAD_DT = "bf16"  # fc head weight dtype ("bf16" | "f32r")

S_EMB = 64.0 if EMB_FP8 else 1.0
S_M = 16.0 if ATTN_FP8 else 1.0
S_WQB = 16.0 if ATTN_FP8 else 1.0
S_WV = 32.0 if V_FP8 else 1.0

E4NP = ml_dtypes.float8_e4m3
BFNP = ml_dtypes.bfloat16


def _round_f32r(x):
    u = np.ascontiguousarray(x, dtype=np.float32).view(np.uint32).copy()
    lsb = (u >> np.uint32(12)) & np.uint32(1)
    u += np.uint32(0x7FF) + lsb
    u &= np.uint32(0xFFFFF000)
    return u.view(np.float32)


def _chunk_major(v, nchunk):
    return np.ascontiguousarray(
        np.asarray(v, dtype=np.float32).reshape(nchunk, 128).T
    )


def _sbuf_layout(w, nk):
    """[nk*128, F] -> [128, nk, F] partition-major image."""
    w = np.asarray(w, dtype=np.float32)
    f = w.shape[1]
    return np.ascontiguousarray(
        w.reshape(nk, 128, f).transpose(1, 0, 2)
    )


def _hilo_rows(v):
    """[N] -> [2, N] bf16 (hi, lo) rows."""
    v = np.asarray(v, np.float32)
    hi = v.astype(BFNP)
    lo = (v - hi.astype(np.float32)).astype(BFNP)
    return np.ascontiguousarray(np.stack([hi, lo], axis=0))


def _cast(x, dt):
    if dt == "e4m3":
        return np.clip(x, -240.0, 240.0).astype(E4NP)
    if dt == "bf16":
        return np.asarray(x, np.float32).astype(BFNP)
    return _round_f32r(x)


def _build(age_scale_f, b23_f, bemb_nz, bv_nz, s_affine_triv=True,
           sim_acts=False):
    import concourse.tile as tile
    import concourse.bass as bass
    from concourse import bacc, mybir

    F32 = mybir.dt.float32
    F32R = mybir.dt.float32r
    BF16 = mybir.dt.bfloat16
    E4M3 = mybir.dt.float8e4
    AF = mybir.ActivationFunctionType
    ALU = mybir.AluOpType
    AX = mybir.AxisListType
    PM = mybir.MatmulPerfMode
    ts = bass.ts
    AF_LRELU = AF.Relu if sim_acts else AF.Lrelu

    I32 = mybir.dt.int32
    RSQRT_MAGIC = 0x5F3759DF

    SDT = E4M3 if (ATTN_FP8 or V_FP8) else BF16
    XDT = E4M3 if EMB_FP8 else BF16
    WEDT = E4M3 if EMB_FP8 else BF16
    MDT = E4M3 if ATTN_FP8 else BF16
    WVDT = E4M3 if V_FP8 else BF16
    ABDT = F32R if ABSC_DT == "f32r" else BF16
    HDT = F32R if HEAD_DT == "f32r" else BF16

    def kch(n, fp8):
        """Chunk iteration: DoubleRow pairs if fp8 else single chunks."""
        if fp8:
            return [
                (slice(2 * p, 2 * p + 2), p == 0, p == n // 2 - 1, PM.DoubleRow)
                for p in range(n // 2)
            ]
        return [(slice(k, k + 1), k == 0, k == n - 1, None) for k in range(n)]

    nc = bacc.Bacc("TRN2", target_bir_lowering=False, debug=False)

    def inp(name, shape, dt):
        return nc.dram_tensor(name, shape, dt, kind="ExternalInput").ap()

    X = inp("x", (NB, 128, 8, C), XDT)
    WEMB = inp("wemb", (128, 8, D), WEDT)
    MQK = inp("m_mat", (128, 8, D), MDT)
    WQB = inp("wqb", (128, 8, C), MDT)
    WV = inp("wv", (128, 8, D), WVDT)
    WP1G = inp("wp1g", (128, 8, 128), BF16)
    WP2 = inp("wp2", (H, 1), BF16)
    WF1 = inp("wf1", (32, 128, 512), HDT)
    W23BC = inp("w23bc", (NB, DF), F32)
    GS = inp("gs_c", (128, 8), F32)
    BS = inp("bs_c", (128, 8), F32)
    BP1E = inp("bp1e", (H, 1), F32)
    BF1R = inp("bf1r", (2, DF), BF16)
    IDENT = inp("ident", (128, 128), BF16)
    BEMB = inp("bemb_row", (1, D), F32R) if bemb_nz else None
    BVR = inp("bv_row", (1, D), F32R) if bv_nz else None
    RUL = nc.dram_tensor("rul", (NB, 1), F32, kind="ExternalOutput").ap()

    with tile.TileContext(nc) as tc:
        # ---- long-lived tiles ----------------------------------------
        glob = tc.alloc_tile_pool(name="glob", bufs=1)
        id_sb = glob.tile([128, 128], BF16, name="id_sb")
        magic_t = glob.tile([128, 4], I32, name="magic_t")
        ages_t = glob.tile([128, 1], F32, name="ages_t")
        ones_c = glob.tile([128, 1], BF16, name="ones_c")
        idf1 = glob.tile([1, 1], F32, name="idf1")
        pool16 = glob.tile([NB, D], BF16, name="pool16")
        ones1 = glob.tile([2, NB], BF16, name="ones1")
        gs_sb = glob.tile([128, 8], F32, name="gs_sb")
        bs_sb = glob.tile([128, 8], F32, name="bs_sb")
        nc.sync.dma_start(id_sb[:], IDENT[:])
        nc.sync.dma_start(gs_sb[:], GS[:])
        nc.sync.dma_start(bs_sb[:], BS[:])
        nc.gpsimd.memset(magic_t[:], RSQRT_MAGIC)
        nc.gpsimd.memset(ages_t[:], age_scale_f)
        nc.gpsimd.memset(ones_c[:], 1.0)
        nc.gpsimd.memset(idf1[:], 1.0)
        nc.gpsimd.memset(ones1[:], 1.0)
        ones_r = None
        if bemb_nz or bv_nz:
            ones_r = glob.tile([1, 128], F32R, name="ones_r")
            nc.gpsimd.memset(ones_r[:], 1.0)
        bemb_sb = None
        if bemb_nz:
            bemb_sb = glob.tile([1, D], F32R, name="bemb_sb")
            nc.sync.dma_start(bemb_sb[:], BEMB[:])
        bv_sb = None
        if bv_nz:
            bv_sb = glob.tile([1, D], F32R, name="bv_sb")
            nc.sync.dma_start(bv_sb[:], BVR[:])

        # ---- weights --------------------------------------------------
        wts = tc.alloc_tile_pool(name="wts", bufs=1)
        wemb_sb = wts.tile([128, 8, D], WEDT, name="wemb_sb")
        m_sb = wts.tile([128, 8, D], MDT, name="m_sb")
        wqb_sb = wts.tile([128, 8, C], MDT, name="wqb_sb")
        wv_sb = wts.tile([128, 8, D], WVDT, name="wv_sb")
        wp1_sb = wts.tile([128, 8, 128], BF16, name="wp1_sb")
        wp2_sb = wts.tile([H, 1], BF16, name="wp2_sb")
        bp1_sb = wts.tile([H, 1], F32, name="bp1_sb")
        bf1r_sb = wts.tile([2, DF], BF16, name="bf1r_sb")
        w23bc_sb = wts.tile([NB, DF], F32, name="w23bc_sb")
        pooledT = wts.tile([128, 8, NB], BF16, name="pooledT")
        weight_dmas = [
            (m_sb, MQK), (wqb_sb, WQB), (wv_sb, WV), (wp1_sb, WP1G),
            (wp2_sb, WP2), (bp1_sb, BP1E), (bf1r_sb, BF1R), (w23bc_sb, W23BC),
        ]

        def emit_rsqrt(pool, v_ap, w, tagp, eps, iters=1):
            """[128,w] -> 1/sqrt(v + eps) elementwise on DVE (Quake+Newton)."""
            ve = pool.tile([128, w], F32, name=f"{tagp}ve", tag=f"{tagp}ve")
            nc.vector.tensor_scalar(ve[:], v_ap, eps, None, op0=ALU.add)
            y = pool.tile([128, w], F32, name=f"{tagp}y0", tag=f"{tagp}y0")
            nc.vector.tensor_scalar(
                y.bitcast(I32)[:], ve.bitcast(I32)[:], 1, None,
                op0=ALU.logical_shift_right,
            )
            nc.vector.scalar_tensor_tensor(
                y.bitcast(I32)[:], y.bitcast(I32)[:], -1, magic_t[:, 0:w],
                op0=ALU.mult, op1=ALU.add,
            )
            for it in range(iters):
                a = pool.tile([128, w], F32, name=f"{tagp}a{it}", tag=f"{tagp}a{it}")
                nc.vector.tensor_tensor(a[:], y[:], y[:], op=ALU.mult)
                nc.vector.tensor_tensor(a[:], a[:], ve[:], op=ALU.mult)
                nc.vector.tensor_scalar(
                    a[:], a[:], -0.5, 1.5, op0=ALU.mult, op1=ALU.add
                )
                nc.vector.tensor_tensor(y[:], y[:], a[:], op=ALU.mult)
            return y

        # ---- pipelined main loop -------------------------------------
        with (
            tc.tile_pool(name="px", bufs=2) as px,
            tc.tile_pool(name="pw3", bufs=16) as pw3,
            tc.tile_pool(name="psen", bufs=1) as psen,
            tc.tile_pool(name="pS", bufs=2) as pS,
            tc.tile_pool(name="pmid", bufs=1) as pmid,
            tc.tile_pool(name="pfn", bufs=3) as pfn,
            tc.tile_pool(name="psc1", bufs=2) as psc1,
            tc.tile_pool(name="psc2", bufs=2) as psc2,
            tc.tile_pool(name="pgel", bufs=1) as pgel,
            tc.tile_pool(name="ps_emb", bufs=3, space="PSUM") as ps_emb,
            tc.tile_pool(name="ps_main", bufs=3, space="PSUM") as ps_main,
            tc.tile_pool(name="ps_small", bufs=2, space="PSUM") as ps_small,
        ):
            st = [dict() for _ in range(NB)]

            def p1_emb(i):
                s = st[i]
                if i == 0:
                    # split first-batch DMAs so ck0/dh0 compute starts early
                    xb = px.tile([128, 8, C], XDT, name="xb", tag="xb")
                    nc.sync.dma_start(xb[:, :, 0:128], X[0][:, :, 0:128])
                    nc.sync.dma_start(
                        wemb_sb[:, :, 0:512], WEMB[:, :, 0:512]
                    )
                    nc.sync.dma_start(xb[:, :, 128:512], X[0][:, :, 128:512])
                    nc.sync.dma_start(
                        wemb_sb[:, :, 512:1024], WEMB[:, :, 512:1024]
                    )
                    for w_t, w_d in weight_dmas:
                        nc.sync.dma_start(w_t[:], w_d[:])
                    s["xb"] = xb
                xb = s.pop("xb")
                if i + 1 < NB:
                    xb2 = px.tile([128, 8, C], XDT, name="xb2", tag="xb")
                    nc.sync.dma_start(xb2[:], X[i + 1])
                    st[i + 1]["xb"] = xb2
                sen_n = psen.tile([128, 4, D], BF16, name="sen_n", tag="sen")
                for ck in range(4):
                    bn6 = psc1.tile([128, 2, 6], F32, name="bn6", tag="st6")
                    ph2 = []
                    for dh in range(2):
                        ps_s = ps_emb.tile([128, 512], F32, name="ps_s", tag="ps_s")
                        for sl, sta, stp, pm in kch(8, EMB_FP8):
                            nc.tensor.matmul(
                                ps_s[:],
                                xb[:, sl, ts(ck, 128)],
                                wemb_sb[:, sl, dh * 512:(dh + 1) * 512],
                                start=sta,
                                stop=(stp and not bemb_nz),
                                perf_mode=pm,
                            )
                        if bemb_nz:
                            nc.tensor.matmul(
                                ps_s[:],
                                ones_r[0:1, :],
                                bemb_sb[0:1, dh * 512:(dh + 1) * 512],
                                start=False, stop=True,
                            )
                        nc.vector.bn_stats(bn6[:, dh, :], ps_s[:])
                        ph2.append(ps_s)
                    bnag = psc1.tile([128, 2], F32, name="bnag", tag="bnag")
                    nc.vector.bn_aggr(bnag[:], bn6[:])
                    i_t = emit_rsqrt(
                        psc1, bnag[:, 1:2], 1, "l1", EPS * S_EMB * S_EMB
                    )
                    negmi = psc1.tile([128, 1], F32, name="negmi", tag="negmi")
                    nc.vector.scalar_tensor_tensor(
                        negmi[:], bnag[:, 0:1], -1.0, i_t[:],
                        op0=ALU.mult, op1=ALU.mult,
                    )
                    for dh in range(2):
                        nc.scalar.activation(
                            sen_n[:, ck, dh * 512:(dh + 1) * 512],
                            ph2[dh][:], AF.Identity,
                            bias=negmi[:], scale=i_t[:],
                        )
                if s_affine_triv:
                    # g_s==1, b_s==0: S_t is a pure transpose -> DMA xbar.
                    # ck-major layout keeps each transpose's output
                    # contiguous (2KB runs) for fast descriptors.
                    S4 = pS.tile([128, 4, 8, 128], SDT, name="S4", tag="S")
                    for ck in range(4):
                        nc.sync.dma_start_transpose(
                            S4[:, ck, :, :], sen_n[:, ck, :]
                        )
                    s["S4"] = S4
                else:
                    s["sen_n"] = sen_n

            def p1_tr(i):
                # fallback path: PE transpose + affine (general g_s/b_s)
                s = st[i]
                if "sen_n" not in s:
                    return
                sen_n = s.pop("sen_n")
                S4 = pS.tile([128, 4, 8, 128], SDT, name="S4", tag="S")
                for dk in range(8):
                    ps_t = ps_small.tile([128, 512], BF16, name="ps_t", tag="sm")
                    for ck in range(4):
                        nc.tensor.transpose(
                            ps_t[:, ts(ck, 128)], sen_n[:, ck, ts(dk, 128)],
                            id_sb[:],
                        )
                    nc.scalar.activation(
                        S4[:, :, dk, :], ps_t[:], AF.Identity,
                        bias=bs_sb[:, dk:dk + 1], scale=gs_sb[:, dk:dk + 1],
                    )
                s["S4"] = S4

            def p2_big(i):
                s = st[i]
                S4 = s.pop("S4")
                # RT = (S M)^T  [e(8), n=C]
                RT = pmid.tile([128, 8, C], MDT, name="RT", tag="RT")
                for ec in range(8):
                    ptr = ps_main.tile([128, C], F32, name="ptr", tag="pm")
                    for sl, sta, stp, pm in kch(8, ATTN_FP8):
                        nc.tensor.matmul(
                            ptr[:], m_sb[:, sl, ts(ec, 128)], S4[:, :, sl, :],
                            start=sta, stop=stp, perf_mode=pm,
                        )
                    nc.scalar.activation(RT[:, ec, :], ptr[:], AF.Copy)
                # ab = S Wqb * isd/s_wqb  [n(4), m=C]
                ab = pmid.tile([128, 4, C], ABDT, name="ab", tag="ab")
                for nk in range(4):
                    pa = ps_main.tile([128, C], F32, name="pa", tag="pm")
                    for sl, sta, stp, pm in kch(8, ATTN_FP8):
                        nc.tensor.matmul(
                            pa[:], S4[:, nk, sl, :], wqb_sb[:, sl, :],
                            start=sta, stop=stp, perf_mode=pm,
                        )
                    nc.scalar.activation(
                        ab[:, nk, :], pa[:], AF.Copy, scale=float(ISD / S_WQB)
                    )
                # sc = R S^T * isd/s_M + age  [n(4), m=C]
                sc = pmid.tile([128, 4, C], ABDT, name="sc", tag="sc")
                for nk in range(4):
                    pa = ps_main.tile([128, C], F32, name="pa2", tag="pm")
                    for sl, sta, stp, pm in kch(8, ATTN_FP8):
                        nc.tensor.matmul(
                            pa[:], RT[:, sl, ts(nk, 128)], S4[:, :, sl, :],
                            start=sta, stop=stp, perf_mode=pm,
                        )
                    nc.scalar.activation(
                        sc[:, nk, :], pa[:], AF.Identity,
                        bias=ages_t[:], scale=float(ISD / S_M),
                    )
                # V = S Wv  [m(4), D]
                V = pmid.tile([128, 4, D], BF16, name="V", tag="V")
                for mk in range(4):
                    for dh in range(2):
                        pv = ps_main.tile([128, 512], F32, name="pv", tag="pm")
                        for sl, sta, stp, pm in kch(8, V_FP8):
                            nc.tensor.matmul(
                                pv[:],
                                S4[:, mk, sl, :],
                                wv_sb[:, sl, dh * 512:(dh + 1) * 512],
                                start=sta,
                                stop=(stp and not bv_nz),
                                perf_mode=pm,
                            )
                        if bv_nz:
                            nc.tensor.matmul(
                                pv[:],
                                ones_r[0:1, :],
                                bv_sb[0:1, dh * 512:(dh + 1) * 512],
                                start=False, stop=True,
                            )
                        nc.vector.tensor_copy(
                            V[:, mk, dh * 512:(dh + 1) * 512], pv[:]
                        )
                s["V"] = V
                # scoresT[k,n] = sum_j sc[j,k] ab[j,n]; exp -> expT (bf16)
                expT = pmid.tile([128, 4, C], BF16, name="expT", tag="expT")
                for kk in range(4):
                    psc = ps_main.tile([128, C], F32, name="psc", tag="pm")
                    for jk in range(4):
                        nc.tensor.matmul(
                            psc[:], sc[:, jk, ts(kk, 128)], ab[:, jk, :],
                            start=(jk == 0), stop=(jk == 3),
                        )
                    nc.scalar.activation(expT[:, kk, :], psc[:], AF.Exp)
                s["expT"] = expT

            def p2_fused(i):
                s = st[i]
                V = s.pop("V")
                expT = s.pop("expT")
                # row sums of exp (over k) as columns per nk + reciprocal
                pssum = ps_small.tile([128, 4], F32, name="pssum", tag="sm")
                for nk in range(4):
                    for kk in range(4):
                        nc.tensor.matmul(
                            pssum[:, nk:nk + 1],
                            expT[:, kk, ts(nk, 128)],
                            ones_c[:],
                            start=(kk == 0), stop=(kk == 3),
                        )
                recips = psc2.tile([128, 4], F32, name="recips", tag="rec")
                nc.vector.reciprocal(recips[:], pssum[:])
                # fused = softmax @ V * isd (LN folded)
                fN = pfn.tile([128, 4, D], BF16, name="fN", tag="fN")
                bn6f = psc2.tile([128, 2, 6], F32, name="bn6f", tag="bn6f")
                bnagf = psc2.tile([128, 2], F32, name="bnagf", tag="bnagf")
                for nk in range(4):
                    pfs = []
                    for dh in range(2):
                        pf = ps_main.tile([128, 512], F32, name="pf", tag="pm")
                        for kk in range(4):
                            nc.tensor.matmul(
                                pf[:],
                                expT[:, kk, ts(nk, 128)],
                                V[:, kk, dh * 512:(dh + 1) * 512],
                                start=(kk == 0), stop=(kk == 3),
                            )
                        nc.vector.bn_stats(bn6f[:, dh, :], pf[:])
                        pfs.append(pf)
                    nc.vector.bn_aggr(bnagf[:], bn6f[:])
                    s_t = psc2.tile([128, 1], F32, name="s_t", tag="s_t")
                    nc.vector.tensor_scalar(
                        s_t[:], recips[:, nk:nk + 1], float(ISD / S_WV), None,
                        op0=ALU.mult,
                    )
                    s2_t = psc2.tile([128, 1], F32, name="s2_t", tag="s2_t")
                    nc.vector.tensor_tensor(s2_t[:], s_t[:], s_t[:], op=ALU.mult)
                    vs_t = psc2.tile([128, 1], F32, name="vs_t", tag="vs_t")
                    nc.vector.scalar_tensor_tensor(
                        vs_t[:], bnagf[:, 1:2], 1.0, s2_t[:],
                        op0=ALU.mult, op1=ALU.mult,
                    )
                    i2_t = emit_rsqrt(psc2, vs_t[:], 1, "l2", EPS)
                    se_t = psc2.tile([128, 1], F32, name="se_t", tag="se_t")
                    nc.vector.tensor_tensor(se_t[:], s_t[:], i2_t[:], op=ALU.mult)
                    be_t = psc2.tile([128, 1], F32, name="be_t", tag="be_t")
                    nc.vector.scalar_tensor_tensor(
                        be_t[:], bnagf[:, 0:1], -1.0, se_t[:],
                        op0=ALU.mult, op1=ALU.mult,
                    )
                    for dh in range(2):
                        nc.scalar.activation(
                            fN[:, nk, dh * 512:(dh + 1) * 512], pfs[dh][:],
                            AF.Identity, bias=be_t[:], scale=se_t[:],
                        )
                # transpose fN -> fNT on the DMA xbar (consumed by p3 next
                # iteration, so the DMA latency is fully hidden); nk-major
                # layout keeps each transpose's output contiguous
                fNT = pmid.tile([128, 4, 8, 128], BF16, name="fNT", tag="fNT")
                for nk in range(4):
                    nc.sync.dma_start_transpose(
                        fNT[:, nk, :, :], fN[:, nk, :]
                    )
                s["fNT"] = fNT
                s["fN"] = fN

            def p3(i):
                s = st[i]
                fNT = s.pop("fNT")
                ph = ps_main.tile([128, C], F32, name="ph", tag="pm")
                for kc in range(8):
                    nc.tensor.matmul(
                        ph[:], wp1_sb[:, kc, :], fNT[:, :, kc, :],
                        start=(kc == 0), stop=(kc == 7),
                    )
                # gelu (tanh formula; Square/Tanh share the Exp table set)
                gx = pgel.tile([H, C], F32, name="gx", tag="gx")
                nc.scalar.activation(gx[:], ph[0:H, :], AF.Identity, bias=bp1_sb[:])
                g2 = pgel.tile([H, C], F32, name="g2", tag="g2")
                nc.scalar.activation(g2[:], gx[:], AF.Square)
                nc.vector.tensor_scalar(
                    g2[:], g2[:], 0.044715 * 0.7978845608028654,
                    0.7978845608028654, op0=ALU.mult, op1=ALU.add,
                )
                nc.vector.tensor_tensor(g2[:], g2[:], gx[:], op=ALU.mult)
                nc.scalar.activation(g2[:], g2[:], AF.Tanh)
                nc.vector.tensor_scalar(g2[:], g2[:], 1.0, None, op0=ALU.add)
                hT = pgel.tile([H, C], BF16, name="hT", tag="hT")
                nc.vector.scalar_tensor_tensor(
                    hT[:], g2[:], 0.5, gx[:], op0=ALU.mult, op1=ALU.mult,
                )
                s["hT"] = hT

            def p4a(i):
                s = st[i]
                hT = s.pop("hT")
                pps = ps_main.tile([1, C], F32, name="pps", tag="pm")
                nc.tensor.matmul(pps[:], wp2_sb[:], hT[:], start=True, stop=True)
                pnm = psc2.tile([1, 1], F32, name="pnm", tag="pnm")
                nc.vector.tensor_reduce(
                    pnm[:], pps[:], axis=AX.X, op=ALU.max, negate=True
                )
                pw = pgel.tile([1, C], BF16, name="pw", tag="row")
                pse = psc2.tile([1, 1], F32, name="pse", tag="pse")
                nc.scalar.activation(
                    pw[:], pps[:], AF.Exp, bias=pnm[:], accum_out=pse[:]
                )
                prc = psc2.tile([1, 1], F32, name="prc", tag="prc")
                nc.vector.reciprocal(prc[:], pse[:])
                pwn = pgel.tile([1, C], F32, name="pwn", tag="rown")
                nc.vector.tensor_scalar(
                    pwn[:], pw[:], prc[0:1, 0:1], None, op0=ALU.mult
                )
                s["pwn"] = pwn

            def p4b(i):
                s = st[i]
                pwn = s.pop("pwn")
                fN = s.pop("fN")
                ppw = ps_small.tile([128, 4], F32, name="ppw", tag="sm")
                for nk in range(4):
                    nc.tensor.transpose(
                        ppw[:, nk:nk + 1], pwn[0:1, ts(nk, 128)], idf1[:]
                    )
                pwc = pgel.tile([128, 4], BF16, name="pwc", tag="pwc")
                nc.scalar.activation(pwc[:], ppw[:], AF.Copy)
                # pooled row (LN_f affine deferred into Wf1'/bf1' host folds)
                prow_sb = pgel.tile([1, D], BF16, name="prow", tag="prow")
                for dh in range(2):
                    prow = ps_small.tile([1, 512], F32, name="prow_ps", tag="sm")
                    for nk in range(4):
                        nc.tensor.matmul(
                            prow[:], pwc[:, nk:nk + 1],
                            fN[:, nk, dh * 512:(dh + 1) * 512],
                            start=(nk == 0), stop=(nk == 3),
                        )
                    nc.vector.tensor_copy(
                        prow_sb[0:1, dh * 512:(dh + 1) * 512], prow[:]
                    )
                nc.sync.dma_start(pool16[i:i + 1, :], prow_sb[0:1, :])

            for i in range(NB + 3):
                if i < NB:
                    p1_emb(i)
                if 1 <= i <= NB:
                    p2_big(i - 1)
                if 3 <= i:
                    p4a(i - 3)
                if 1 <= i <= NB:
                    p2_fused(i - 1)
                if 3 <= i:
                    p4b(i - 3)
                if i < NB:
                    p1_tr(i)
                if 2 <= i <= NB + 1:
                    p3(i - 2)

            # =================== fc head ==============================
            # Wf2 @ Wf3 folded host-side (no nonlinearity between them):
            # rul = |lrelu(pooled @ Wf1' + bf1') @ W23 + b23|
            wts1 = []
            for t in range(32):
                wt = pw3.tile([128, 512], HDT, name="wt1", tag="w3")
                nc.sync.dma_start(wt[:], WF1[t])
                wts1.append(wt)
            # pooledT [d-part, batch] from pool16 [batch, d]
            for dk in range(8):
                ptp = ps_small.tile([128, NB], BF16, name="ptp", tag="sm")
                nc.tensor.transpose(
                    ptp[:], pool16[0:NB, ts(dk, 128)], id_sb[0:NB, 0:NB]
                )
                nc.scalar.activation(pooledT[:, dk, :], ptp[:], AF.Copy)
            # h1 = lrelu(pooled @ Wf1' + bf1')  [batch-part 16, DF]
            h1 = pgel.tile([NB, DF], BF16, name="h1", tag="h1")
            for fg in range(4):
                hg = ps_main.tile([NB, 512], F32, name="hg", tag="pm")
                for kd in range(8):
                    nc.tensor.matmul(
                        hg[:], pooledT[:, kd, :], wts1[fg * 8 + kd][:],
                        start=(kd == 0), stop=False,
                    )
                nc.tensor.matmul(
                    hg[:], ones1[0:2, :], bf1r_sb[0:2, ts(fg, 512)],
                    start=False, stop=True,
                )
                nc.scalar.activation(
                    h1[0:NB, ts(fg, 512)], hg[:], AF_LRELU, alpha=0.01
                )
            # rul = |h1 . W23 + b23| as a per-partition DVE dot (batch-part)
            rsum4 = psc2.tile([NB, 4], F32, name="rsum4", tag="rsum4")
            for fg in range(4):
                hm = pgel.tile([NB, 512], F32, name="hm", tag="hm")
                nc.vector.tensor_tensor(
                    hm[:], h1[0:NB, ts(fg, 512)], w23bc_sb[0:NB, ts(fg, 512)],
                    op=ALU.mult,
                )
                nc.vector.tensor_reduce(
                    rsum4[:, fg:fg + 1], hm[:], axis=AX.X, op=ALU.add
                )
            rsum = psc2.tile([NB, 1], F32, name="rsum", tag="rsum")
            nc.vector.tensor_reduce(rsum[:], rsum4[:], axis=AX.X, op=ALU.add)
            bf3_t = psc2.tile([NB, 1], F32, name="bf3_t", tag="bf3")
            nc.gpsimd.memset(bf3_t[:], b23_f)
            rul_sb = psc2.tile([NB, 1], F32, name="rul_sb", tag="rul")
            nc.scalar.activation(rul_sb[:], rsum[:], AF.Abs, bias=bf3_t[:])
            nc.sync.dma_start(RUL[:], rul_sb[:])

        wts.release()
        glob.release()

    nc.compile()
    return nc


def _prep_in_maps(inputs):
    f32 = np.float32
    x_enc = np.asarray(inputs["x_enc"], f32)
    W_emb = np.asarray(inputs["W_emb"], f32)
    b_emb = np.asarray(inputs["b_emb"], f32)
    g_s = np.asarray(inputs["g_s"], f32)
    b_s = np.asarray(inputs["b_s"], f32)
    basis = np.asarray(inputs["basis"], np.float64)
    Wq = np.asarray(inputs["Wq"], np.float64)
    bq = np.asarray(inputs["bq"], f32)
    Wk = np.asarray(inputs["Wk"], np.float64)
    bk = np.asarray(inputs["bk"], f32)
    Wv = np.asarray(inputs["Wv"], f32)
    bv = np.asarray(inputs["bv"], f32)
    g_f = np.asarray(inputs["g_f"], f32)
    b_f = np.asarray(inputs["b_f"], f32)
    Wp1 = np.asarray(inputs["Wp1"], f32)
    bp1 = np.asarray(inputs["bp1"], f32)
    Wp2 = np.asarray(inputs["Wp2"], f32)
    Wf1 = np.asarray(inputs["Wf1"], np.float64)
    bf1 = np.asarray(inputs["bf1"], np.float64)
    Wf2 = np.asarray(inputs["Wf2"], np.float64)
    bf2 = np.asarray(inputs["bf2"], np.float64)
    Wf3 = np.asarray(inputs["Wf3"], np.float64)
    b_f64 = np.asarray(inputs["b_f"], np.float64)
    g_f64 = np.asarray(inputs["g_f"], np.float64)

    assert not (np.any(bq) or np.any(bk)), "folded QK path requires bq=bk=0"

    M = (Wq @ Wk.T).astype(f32)
    Wqb = (Wq @ basis.T).astype(f32)

    wp1g = np.zeros((D, 128), f32)
    wp1g[:, :H] = g_f[:, None] * Wp1
    bp1e = (b_f @ Wp1 + bp1).reshape(H, 1).astype(f32)

    # head folds: LN_f affine into Wf1/bf1; Wf2@Wf3 into one vector
    Wf1g = (g_f64[:, None] * Wf1).astype(f32)
    bf1g = (bf1 + b_f64 @ Wf1).astype(f32)
    W23 = (Wf2 @ Wf3[:, 0]).astype(f32)
    b23 = float(bf2 @ Wf3[:, 0])

    e_dt = "e4m3" if EMB_FP8 else "bf16"
    m_dt = "e4m3" if ATTN_FP8 else "bf16"
    v_dt = "e4m3" if V_FP8 else "bf16"
    h_dt = "bf16" if HEAD_DT == "bf16" else "f32r"

    common = {
        "wemb": _cast(_sbuf_layout(W_emb * S_EMB, 8), e_dt),
        "m_mat": _cast(_sbuf_layout(M * S_M, 8), m_dt),
        "wqb": _cast(_sbuf_layout(Wqb * S_WQB, 8), m_dt),
        "wv": _cast(_sbuf_layout(Wv * S_WV, 8), v_dt),
        "wp1g": _cast(_sbuf_layout(wp1g, 8), "bf16"),
        "wp2": Wp2.astype(BFNP),
        "wf1": _cast(
            Wf1g.reshape(8, 128, 4, 512).transpose(2, 0, 1, 3).reshape(32, 128, 512),
            h_dt,
        ),
        "w23bc": np.ascontiguousarray(np.tile(W23.reshape(1, DF), (NB, 1))),
        "gs_c": _chunk_major(g_s, 8),
        "bs_c": _chunk_major(b_s, 8),
        "bp1e": bp1e,
        "bf1r": _hilo_rows(bf1g),
        "ident": np.eye(128).astype(BFNP),
    }
    bemb_nz = bool(np.any(b_emb))
    bv_nz = bool(np.any(bv))
    if bemb_nz:
        common["bemb_row"] = _round_f32r(b_emb.reshape(1, D) * S_EMB)
    if bv_nz:
        common["bv_row"] = _round_f32r(bv.reshape(1, D) * S_WV)

    in_maps = []
    for c in range(NCORES):
        m = dict(common)
        xs = x_enc[c * NB:(c + 1) * NB]
        m["x"] = _cast(
            xs.reshape(NB, 8, 128, C).transpose(0, 2, 1, 3), e_dt
        )
        in_maps.append(m)

    age_scale_f = float(np.asarray(inputs["age_scale"], f32))
    bf3_f = float(np.asarray(inputs["bf3"], np.float64).reshape(-1)[0])
    b23_f = b23 + bf3_f
    s_triv = bool(np.allclose(g_s, 1.0) and not np.any(b_s))
    return in_maps, age_scale_f, b23_f, bemb_nz, bv_nz, s_triv


_NC_CACHE = {}


def build_program(inputs, sim_acts=False):
    in_maps, age_scale_f, b23_f, bemb_nz, bv_nz, s_triv = _prep_in_maps(inputs)
    key = (age_scale_f, b23_f, bemb_nz, bv_nz, s_triv, sim_acts)
    if key not in _NC_CACHE:
        _NC_CACHE[key] = _build(
            age_scale_f, b23_f, bemb_nz, bv_nz, s_triv, sim_acts
        )
    return _NC_CACHE[key], in_maps


def kernel(**inputs):
    from concourse.bass_utils import run_bass_kernel_spmd

    nc, in_maps = build_program(inputs)
    res = run_bass_kernel_spmd(nc, in_maps, core_ids=list(range(NCORES)))
    out = np.concatenate(
        [res.results[c]["rul"] for c in range(NCORES)], axis=0
    ).astype(np.float32)
    return out



# revision 56
# speedup vs baseline: 1.0856x; 1.0367x over previous
"""Trainium2 Bass kernel for nn_CLIP_69458211111620 (v3: folded head).

Data-parallel over batch B=128 across 8 NeuronCores (16 batches/core).
Single fused pass per batch (no DRAM staging), software-pipelined 4 deep:
  P1(i): emb matmuls + LN + transpose -> S
  P2(i-1): RT/ab/sc/V/scoresT matmuls, exp (pre-transposed softmax), fused, LN
  P3(i-2): fNT transpose, pooling MLP hT
  P4(i-3): pool softmax + pooled row (batch-partition layout)
then fc head: Wf2@Wf3 is host-folded to a single vector (no nonlinearity
between them), so the head is pooled @ Wf1' -> lrelu -> @ W23 with
LN_f's affine folded into Wf1'/bf1' as well.

Precision: bf16 matmuls, f32r where critical (fp8 on the attention chain
exceeds the 2e-2 budget per earlier ablations).
"""
import sys

sys.path.insert(0, "/opt/trn_rl_repo")

import numpy as np
import ml_dtypes

NCORES = 8
NB = 16          # batches per core
T, C, D, DF, H = 1024, 512, 1024, 2048, 64
ISD = 1.0 / 32.0  # 1/sqrt(D)
EPS = 1e-5

# ---- precision config (validated by sim.py ablations) -----------------
# fp8 e4m3 on any attention-chain operand exceeds the 2e-2 budget (the
# softmax sharply amplifies quantization noise); all-bf16 sims at 6.6e-3.
EMB_FP8 = False   # x, wemb e4m3 (DoubleRow)
ATTN_FP8 = False  # S, M, wqb e4m3 -> RT/ab/sc DoubleRow
V_FP8 = False     # wv e4m3 (uses e4m3 S) -> V matmul DoubleRow
ABSC_DT = "bf16"  # scores matmul operand dtype ("bf16" | "f32r")
HEAD_DT = "bf16"  # fc head weight dtype ("bf16" | "f32r")

S_EMB = 64.0 if EMB_FP8 else 1.0
S_M = 16.0 if ATTN_FP8 else 1.0
S_WQB = 16.0 if ATTN_FP8 else 1.0
S_WV = 32.0 if V_FP8 else 1.0

E4NP = ml_dtypes.float8_e4m3
BFNP = ml_dtypes.bfloat16


def _round_f32r(x):
    u = np.ascontiguousarray(x, dtype=np.float32).view(np.uint32).copy()
    lsb = (u >> np.uint32(12)) & np.uint32(1)
    u += np.uint32(0x7FF) + lsb
    u &= np.uint32(0xFFFFF000)
    return u.view(np.float32)


def _chunk_major(v, nchunk):
    return np.ascontiguousarray(
        np.asarray(v, dtype=np.float32).reshape(nchunk, 128).T
    )


def _sbuf_layout(w, nk):
    """[nk*128, F] -> [128, nk, F] partition-major image."""
    w = np.asarray(w, dtype=np.float32)
    f = w.shape[1]
    return np.ascontiguousarray(
        w.reshape(nk, 128, f).transpose(1, 0, 2)
    )


def _hilo_rows(v):
    """[N] -> [2, N] bf16 (hi, lo) rows."""
    v = np.asarray(v, np.float32)
    hi = v.astype(BFNP)
    lo = (v - hi.astype(np.float32)).astype(BFNP)
    return np.ascontiguousarray(np.stack([hi, lo], axis=0))


def _cast(x, dt):
    if dt == "e4m3":
        return np.clip(x, -240.0, 240.0).astype(E4NP)
    if dt == "bf16":
        return np.asarray(x, np.float32).astype(BFNP)
    return _round_f32r(x)


def _build(age_scale_f, b23_f, bemb_nz, bv_nz, s_affine_triv=True,
           sim_acts=False):
    import concourse.tile as tile
    import concourse.bass as bass
    from concourse import bacc, mybir

    F32 = mybir.dt.float32
    F32R = mybir.dt.float32r
    BF16 = mybir.dt.bfloat16
    E4M3 = mybir.dt.float8e4
    AF = mybir.ActivationFunctionType
    ALU = mybir.AluOpType
    AX = mybir.AxisListType
    PM = mybir.MatmulPerfMode
    ts = bass.ts
    AF_LRELU = AF.Relu if sim_acts else AF.Lrelu

    I32 = mybir.dt.int32
    RSQRT_MAGIC = 0x5F3759DF

    SDT = E4M3 if (ATTN_FP8 or V_FP8) else BF16
    XDT = E4M3 if EMB_FP8 else BF16
    WEDT = E4M3 if EMB_FP8 else BF16
    MDT = E4M3 if ATTN_FP8 else BF16
    WVDT = E4M3 if V_FP8 else BF16
    ABDT = F32R if ABSC_DT == "f32r" else BF16
    HDT = F32R if HEAD_DT == "f32r" else BF16

    def kch(n, fp8):
        """Chunk iteration: DoubleRow pairs if fp8 else single chunks."""
        if fp8:
            return [
                (slice(2 * p, 2 * p + 2), p == 0, p == n // 2 - 1, PM.DoubleRow)
                for p in range(n // 2)
            ]
        return [(slice(k, k + 1), k == 0, k == n - 1, None) for k in range(n)]

    nc = bacc.Bacc("TRN2", target_bir_lowering=False, debug=False)

    def inp(name, shape, dt):
        return nc.dram_tensor(name, shape, dt, kind="ExternalInput").ap()

    X = inp("x", (NB, 128, 8, C), XDT)
    WEMB = inp("wemb", (128, 8, D), WEDT)
    MQK = inp("m_mat", (128, 8, D), MDT)
    WQB = inp("wqb", (128, 8, C), MDT)
    WV = inp("wv", (128, 8, D), WVDT)
    WP1G = inp("wp1g", (128, 8, 128), BF16)
    WP2 = inp("wp2", (H, 1), BF16)
    WF1 = inp("wf1", (32, 128, 512), HDT)
    W23BC = inp("w23bc", (NB, DF), BF16)
    GS = inp("gs_c", (128, 8), F32)
    BS = inp("bs_c", (128, 8), F32)
    BP1E = inp("bp1e", (H, 1), F32)
    BF1R = inp("bf1r", (1, DF), BF16)
    IDENT = inp("ident", (128, 128), BF16)
    BEMB = inp("bemb_row", (1, D), F32R) if bemb_nz else None
    BVR = inp("bv_row", (1, D), F32R) if bv_nz else None
    RUL = nc.dram_tensor("rul", (NB, 1), F32, kind="ExternalOutput").ap()

    with tile.TileContext(nc) as tc:
        # ---- long-lived tiles ----------------------------------------
        glob = tc.alloc_tile_pool(name="glob", bufs=1)
        id_sb = glob.tile([128, 128], BF16, name="id_sb")
        magic_t = glob.tile([128, 4], I32, name="magic_t")
        ages_t = glob.tile([128, 1], F32, name="ages_t")
        ones_c = glob.tile([128, 1], BF16, name="ones_c")
        idf1 = glob.tile([1, 1], F32, name="idf1")
        pool16 = glob.tile([NB, D], BF16, name="pool16")
        ones1 = glob.tile([1, NB], BF16, name="ones1")
        gs_sb = glob.tile([128, 8], F32, name="gs_sb")
        bs_sb = glob.tile([128, 8], F32, name="bs_sb")
        nc.sync.dma_start(id_sb[:], IDENT[:])
        nc.sync.dma_start(gs_sb[:], GS[:])
        nc.sync.dma_start(bs_sb[:], BS[:])
        nc.gpsimd.memset(magic_t[:], RSQRT_MAGIC)
        nc.gpsimd.memset(ages_t[:], age_scale_f)
        nc.gpsimd.memset(ones_c[:], 1.0)
        nc.gpsimd.memset(idf1[:], 1.0)
        nc.gpsimd.memset(ones1[:], 1.0)
        ones_r = None
        if bemb_nz or bv_nz:
            ones_r = glob.tile([1, 128], F32R, name="ones_r")
            nc.gpsimd.memset(ones_r[:], 1.0)
        bemb_sb = None
        if bemb_nz:
            bemb_sb = glob.tile([1, D], F32R, name="bemb_sb")
            nc.sync.dma_start(bemb_sb[:], BEMB[:])
        bv_sb = None
        if bv_nz:
            bv_sb = glob.tile([1, D], F32R, name="bv_sb")
            nc.sync.dma_start(bv_sb[:], BVR[:])

        # ---- weights --------------------------------------------------
        wts = tc.alloc_tile_pool(name="wts", bufs=1)
        wemb_sb = wts.tile([128, 8, D], WEDT, name="wemb_sb")
        m_sb = wts.tile([128, 8, D], MDT, name="m_sb")
        wqb_sb = wts.tile([128, 8, C], MDT, name="wqb_sb")
        wv_sb = wts.tile([128, 8, D], WVDT, name="wv_sb")
        wp1_sb = wts.tile([128, 8, 128], BF16, name="wp1_sb")
        wp2_sb = wts.tile([H, 1], BF16, name="wp2_sb")
        bp1_sb = wts.tile([H, 1], F32, name="bp1_sb")
        bf1r_sb = wts.tile([1, DF], BF16, name="bf1r_sb")
        w23bc_sb = wts.tile([NB, DF], BF16, name="w23bc_sb")
        pooledT = wts.tile([128, 8, NB], BF16, name="pooledT")
        weight_dmas = [
            (m_sb, MQK), (wqb_sb, WQB), (wv_sb, WV), (wp1_sb, WP1G),
            (wp2_sb, WP2), (bp1_sb, BP1E), (bf1r_sb, BF1R), (w23bc_sb, W23BC),
        ]

        def emit_rsqrt(pool, v_ap, w, tagp, eps, iters=1, v_tile=None):
            """[128,w] -> 1/sqrt(v + eps) elementwise on DVE (Quake+Newton).

            eps=None: pass v_tile (a [128,w] tile already including eps);
            skips the eps-add op.
            """
            if eps is None:
                ve = v_tile
            else:
                ve = pool.tile([128, w], F32, name=f"{tagp}ve", tag=f"{tagp}ve")
                nc.vector.tensor_scalar(ve[:], v_ap, eps, None, op0=ALU.add)
            y = pool.tile([128, w], F32, name=f"{tagp}y0", tag=f"{tagp}y0")
            nc.vector.tensor_scalar(
                y.bitcast(I32)[:], ve.bitcast(I32)[:], 1, None,
                op0=ALU.logical_shift_right,
            )
            nc.vector.scalar_tensor_tensor(
                y.bitcast(I32)[:], y.bitcast(I32)[:], -1, magic_t[:, 0:w],
                op0=ALU.mult, op1=ALU.add,
            )
            for it in range(iters):
                a = pool.tile([128, w], F32, name=f"{tagp}a{it}", tag=f"{tagp}a{it}")
                nc.vector.tensor_tensor(a[:], y[:], y[:], op=ALU.mult)
                nc.vector.tensor_tensor(a[:], a[:], ve[:], op=ALU.mult)
                nc.vector.tensor_scalar(
                    a[:], a[:], -0.5, 1.5, op0=ALU.mult, op1=ALU.add
                )
                nc.vector.tensor_tensor(y[:], y[:], a[:], op=ALU.mult)
            return y

        # ---- pipelined main loop -------------------------------------
        with (
            tc.tile_pool(name="px", bufs=2) as px,
            tc.tile_pool(name="pw3", bufs=14) as pw3,
            tc.tile_pool(name="psen", bufs=2) as psen,
            tc.tile_pool(name="pS", bufs=2) as pS,
            tc.tile_pool(name="pmid", bufs=1) as pmid,
            tc.tile_pool(name="pfn", bufs=3) as pfn,
            tc.tile_pool(name="psc1", bufs=2) as psc1,
            tc.tile_pool(name="psc2", bufs=2) as psc2,
            tc.tile_pool(name="pgel", bufs=1) as pgel,
            tc.tile_pool(name="ps_emb", bufs=3, space="PSUM") as ps_emb,
            tc.tile_pool(name="ps_main", bufs=3, space="PSUM") as ps_main,
            tc.tile_pool(name="ps_small", bufs=2, space="PSUM") as ps_small,
        ):
            st = [dict() for _ in range(NB)]

            def p1_emb(i):
                s = st[i]
                if i == 0:
                    # split first-batch DMAs so ck0/dh0 compute starts early
                    xb = px.tile([128, 8, C], XDT, name="xb", tag="xb")
                    nc.sync.dma_start(xb[:, :, 0:128], X[0][:, :, 0:128])
                    nc.sync.dma_start(
                        wemb_sb[:, :, 0:512], WEMB[:, :, 0:512]
                    )
                    nc.sync.dma_start(xb[:, :, 128:512], X[0][:, :, 128:512])
                    nc.sync.dma_start(
                        wemb_sb[:, :, 512:1024], WEMB[:, :, 512:1024]
                    )
                    for w_t, w_d in weight_dmas:
                        nc.sync.dma_start(w_t[:], w_d[:])
                    s["xb"] = xb
                xb = s.pop("xb")
                if i + 1 < NB:
                    xb2 = px.tile([128, 8, C], XDT, name="xb2", tag="xb")
                    nc.sync.dma_start(xb2[:], X[i + 1])
                    st[i + 1]["xb"] = xb2
                # MM + fast Copy evacuation only; LN stats are deferred to
                # p1_stats so the DVE queue serves p2_fused's chain first
                sen_r = psen.tile([128, 4, D], BF16, name="sen_r", tag="sen")
                for ck in range(4):
                    for dh in range(2):
                        ps_s = ps_emb.tile([128, 512], F32, name="ps_s", tag="ps_s")
                        for sl, sta, stp, pm in kch(8, EMB_FP8):
                            nc.tensor.matmul(
                                ps_s[:],
                                xb[:, sl, ts(ck, 128)],
                                wemb_sb[:, sl, dh * 512:(dh + 1) * 512],
                                start=sta,
                                stop=(stp and not bemb_nz),
                                perf_mode=pm,
                            )
                        if bemb_nz:
                            nc.tensor.matmul(
                                ps_s[:],
                                ones_r[0:1, :],
                                bemb_sb[0:1, dh * 512:(dh + 1) * 512],
                                start=False, stop=True,
                            )
                        nc.scalar.activation(
                            sen_r[:, ck, dh * 512:(dh + 1) * 512],
                            ps_s[:], AF.Copy,
                        )
                s["sen_r"] = sen_r

            def p1_stats(i):
                s = st[i]
                sen_r = s.pop("sen_r")
                for ck in range(4):
                    bn6 = psc1.tile([128, 2, 6], F32, name="bn6", tag="st6")
                    for dh in range(2):
                        nc.vector.bn_stats(
                            bn6[:, dh, :],
                            sen_r[:, ck, dh * 512:(dh + 1) * 512],
                        )
                    bnag = psc1.tile([128, 2], F32, name="bnag", tag="bnag")
                    nc.vector.bn_aggr(bnag[:], bn6[:])
                    i_t = emit_rsqrt(
                        psc1, bnag[:, 1:2], 1, "l1", EPS * S_EMB * S_EMB
                    )
                    negmi = psc1.tile([128, 1], F32, name="negmi", tag="negmi")
                    nc.vector.scalar_tensor_tensor(
                        negmi[:], bnag[:, 0:1], -1.0, i_t[:],
                        op0=ALU.mult, op1=ALU.mult,
                    )
                    for dh in range(2):
                        # normalize in place (elementwise, same AP in/out)
                        nc.scalar.activation(
                            sen_r[:, ck, dh * 512:(dh + 1) * 512],
                            sen_r[:, ck, dh * 512:(dh + 1) * 512],
                            AF.Identity, bias=negmi[:], scale=i_t[:],
                        )
                if s_affine_triv:
                    # g_s==1, b_s==0: S_t is a pure transpose -> DMA xbar.
                    # ck-major layout keeps each transpose's output
                    # contiguous (2KB runs) for fast descriptors.
                    S4 = pS.tile([128, 4, 8, 128], SDT, name="S4", tag="S")
                    for ck in range(4):
                        nc.sync.dma_start_transpose(
                            S4[:, ck, :, :], sen_r[:, ck, :]
                        )
                    s["S4"] = S4
                else:
                    s["sen_n"] = sen_r

            def p1_tr(i):
                # fallback path: PE transpose + affine (general g_s/b_s)
                s = st[i]
                if "sen_n" not in s:
                    return
                sen_n = s.pop("sen_n")
                S4 = pS.tile([128, 4, 8, 128], SDT, name="S4", tag="S")
                for dk in range(8):
                    ps_t = ps_small.tile([128, 512], BF16, name="ps_t", tag="sm")
                    for ck in range(4):
                        nc.tensor.transpose(
                            ps_t[:, ts(ck, 128)], sen_n[:, ck, ts(dk, 128)],
                            id_sb[:],
                        )
                    nc.scalar.activation(
                        S4[:, :, dk, :], ps_t[:], AF.Identity,
                        bias=bs_sb[:, dk:dk + 1], scale=gs_sb[:, dk:dk + 1],
                    )
                s["S4"] = S4

            def p2_big(i):
                s = st[i]
                S4 = s.pop("S4")
                # RT = (S M)^T  [e(8), n=C]
                RT = pmid.tile([128, 8, C], MDT, name="RT", tag="RT")
                for ec in range(8):
                    ptr = ps_main.tile([128, C], F32, name="ptr", tag="pm")
                    for sl, sta, stp, pm in kch(8, ATTN_FP8):
                        nc.tensor.matmul(
                            ptr[:], m_sb[:, sl, ts(ec, 128)], S4[:, :, sl, :],
                            start=sta, stop=stp, perf_mode=pm,
                        )
                    nc.scalar.activation(RT[:, ec, :], ptr[:], AF.Copy)
                # ab = S Wqb * isd/s_wqb  [n(4), m=C]
                ab = pmid.tile([128, 4, C], ABDT, name="ab", tag="ab")
                for nk in range(4):
                    pa = ps_main.tile([128, C], F32, name="pa", tag="pm")
                    for sl, sta, stp, pm in kch(8, ATTN_FP8):
                        nc.tensor.matmul(
                            pa[:], S4[:, nk, sl, :], wqb_sb[:, sl, :],
                            start=sta, stop=stp, perf_mode=pm,
                        )
                    nc.scalar.activation(
                        ab[:, nk, :], pa[:], AF.Copy, scale=float(ISD / S_WQB)
                    )
                # sc = R S^T * isd/s_M + age  [n(4), m=C]
                sc = pmid.tile([128, 4, C], ABDT, name="sc", tag="sc")
                for nk in range(4):
                    pa = ps_main.tile([128, C], F32, name="pa2", tag="pm")
                    for sl, sta, stp, pm in kch(8, ATTN_FP8):
                        nc.tensor.matmul(
                            pa[:], RT[:, sl, ts(nk, 128)], S4[:, :, sl, :],
                            start=sta, stop=stp, perf_mode=pm,
                        )
                    nc.scalar.activation(
                        sc[:, nk, :], pa[:], AF.Identity,
                        bias=ages_t[:], scale=float(ISD / S_M),
                    )
                # V = S Wv  [m(4), D]
                V = pmid.tile([128, 4, D], BF16, name="V", tag="V")
                for mk in range(4):
                    for dh in range(2):
                        pv = ps_main.tile([128, 512], F32, name="pv", tag="pm")
                        for sl, sta, stp, pm in kch(8, V_FP8):
                            nc.tensor.matmul(
                                pv[:],
                                S4[:, mk, sl, :],
                                wv_sb[:, sl, dh * 512:(dh + 1) * 512],
                                start=sta,
                                stop=(stp and not bv_nz),
                                perf_mode=pm,
                            )
                        if bv_nz:
                            nc.tensor.matmul(
                                pv[:],
                                ones_r[0:1, :],
                                bv_sb[0:1, dh * 512:(dh + 1) * 512],
                                start=False, stop=True,
                            )
                        nc.vector.tensor_copy(
                            V[:, mk, dh * 512:(dh + 1) * 512], pv[:]
                        )
                s["V"] = V
                # scoresT[k,n] = sum_j sc[j,k] ab[j,n]; exp -> expT (bf16)
                expT = pmid.tile([128, 4, C], BF16, name="expT", tag="expT")
                for kk in range(4):
                    psc = ps_main.tile([128, C], F32, name="psc", tag="pm")
                    for jk in range(4):
                        nc.tensor.matmul(
                            psc[:], sc[:, jk, ts(kk, 128)], ab[:, jk, :],
                            start=(jk == 0), stop=(jk == 3),
                        )
                    nc.scalar.activation(expT[:, kk, :], psc[:], AF.Exp)
                s["expT"] = expT

            def p2_fused(i):
                s = st[i]
                V = s.pop("V")
                expT = s.pop("expT")
                # row sums of exp (over k) as columns per nk
                pssum_ps = ps_small.tile([128, 4], F32, name="pssum", tag="sm")
                for nk in range(4):
                    for kk in range(4):
                        nc.tensor.matmul(
                            pssum_ps[:, nk:nk + 1],
                            expT[:, kk, ts(nk, 128)],
                            ones_c[:],
                            start=(kk == 0), stop=(kk == 3),
                        )
                pssum = psc2.tile([128, 4], F32, name="pssums", tag="rec")
                nc.vector.tensor_copy(pssum[:], pssum_ps[:])
                # fused LN via scale-invariance: LN(c*x) with c=isd/denom
                # equals (x - mu_x) * rsqrt(var_x + eps/c^2); eps/c^2 =
                # eps * (S_WV/isd)^2 * denom^2 folds into one stt op.
                EPSC = float(EPS * (S_WV / ISD) ** 2)
                fN = pfn.tile([128, 4, D], BF16, name="fN", tag="fN")
                bn6f = psc2.tile([128, 2, 6], F32, name="bn6f", tag="bn6f")
                bnagf = psc2.tile([128, 2], F32, name="bnagf", tag="bnagf")
                for nk in range(4):
                    pfs = []
                    for dh in range(2):
                        pf = ps_main.tile([128, 512], F32, name="pf", tag="pm")
                        for kk in range(4):
                            nc.tensor.matmul(
                                pf[:],
                                expT[:, kk, ts(nk, 128)],
                                V[:, kk, dh * 512:(dh + 1) * 512],
                                start=(kk == 0), stop=(kk == 3),
                            )
                        nc.vector.bn_stats(bn6f[:, dh, :], pf[:])
                        pfs.append(pf)
                    nc.vector.bn_aggr(bnagf[:], bn6f[:])
                    vs_t = psc2.tile([128, 1], F32, name="vs_t", tag="vs_t")
                    nc.vector.scalar_tensor_tensor(
                        vs_t[:], pssum[:, nk:nk + 1], EPSC, pssum[:, nk:nk + 1],
                        op0=ALU.mult, op1=ALU.mult,
                    )
                    nc.vector.tensor_tensor(
                        vs_t[:], vs_t[:], bnagf[:, 1:2], op=ALU.add
                    )
                    i2_t = emit_rsqrt(psc2, None, 1, "l2", None, v_tile=vs_t)
                    be_t = psc2.tile([128, 1], F32, name="be_t", tag="be_t")
                    nc.vector.scalar_tensor_tensor(
                        be_t[:], bnagf[:, 0:1], -1.0, i2_t[:],
                        op0=ALU.mult, op1=ALU.mult,
                    )
                    for dh in range(2):
                        nc.scalar.activation(
                            fN[:, nk, dh * 512:(dh + 1) * 512], pfs[dh][:],
                            AF.Identity, bias=be_t[:], scale=i2_t[:],
                        )
                # transpose fN -> fNT on the DMA xbar (consumed by p3 next
                # iteration, so the DMA latency is fully hidden); nk-major
                # layout keeps each transpose's output contiguous
                fNT = pmid.tile([128, 4, 8, 128], BF16, name="fNT", tag="fNT")
                for nk in range(4):
                    nc.sync.dma_start_transpose(
                        fNT[:, nk, :, :], fN[:, nk, :]
                    )
                s["fNT"] = fNT
                s["fN"] = fN

            def p3(i):
                s = st[i]
                fNT = s.pop("fNT")
                ph = ps_main.tile([128, C], F32, name="ph", tag="pm")
                for kc in range(8):
                    nc.tensor.matmul(
                        ph[:], wp1_sb[:, kc, :], fNT[:, :, kc, :],
                        start=(kc == 0), stop=(kc == 7),
                    )
                # gelu (tanh formula; Square/Tanh share the Exp table set)
                gx = pgel.tile([H, C], BF16, name="gx", tag="gx")
                nc.scalar.activation(gx[:], ph[0:H, :], AF.Identity, bias=bp1_sb[:])
                g2 = pgel.tile([H, C], BF16, name="g2", tag="g2")
                nc.scalar.activation(g2[:], gx[:], AF.Square)
                nc.vector.tensor_scalar(
                    g2[:], g2[:], 0.044715 * 0.7978845608028654,
                    0.7978845608028654, op0=ALU.mult, op1=ALU.add,
                )
                nc.vector.tensor_tensor(g2[:], g2[:], gx[:], op=ALU.mult)
                nc.scalar.activation(g2[:], g2[:], AF.Tanh)
                nc.vector.tensor_scalar(g2[:], g2[:], 1.0, None, op0=ALU.add)
                hT = pgel.tile([H, C], BF16, name="hT", tag="hT")
                nc.vector.scalar_tensor_tensor(
                    hT[:], g2[:], 0.5, gx[:], op0=ALU.mult, op1=ALU.mult,
                )
                s["hT"] = hT

            def p4a(i):
                s = st[i]
                hT = s.pop("hT")
                pps = ps_main.tile([1, C], F32, name="pps", tag="pm")
                nc.tensor.matmul(pps[:], wp2_sb[:], hT[:], start=True, stop=True)
                pnm = psc2.tile([1, 1], F32, name="pnm", tag="pnm")
                nc.vector.tensor_reduce(
                    pnm[:], pps[:], axis=AX.X, op=ALU.max, negate=True
                )
                pw = pgel.tile([1, C], BF16, name="pw", tag="row")
                pse = psc2.tile([1, 1], F32, name="pse", tag="pse")
                nc.scalar.activation(
                    pw[:], pps[:], AF.Exp, bias=pnm[:], accum_out=pse[:]
                )
                prc = psc2.tile([1, 1], F32, name="prc", tag="prc")
                nc.vector.reciprocal(prc[:], pse[:])
                pwn = pgel.tile([1, C], F32, name="pwn", tag="rown")
                nc.vector.tensor_scalar(
                    pwn[:], pw[:], prc[0:1, 0:1], None, op0=ALU.mult
                )
                s["pwn"] = pwn

            def p4b(i):
                s = st[i]
                pwn = s.pop("pwn")
                fN = s.pop("fN")
                ppw = ps_small.tile([128, 4], F32, name="ppw", tag="sm")
                for nk in range(4):
                    nc.tensor.transpose(
                        ppw[:, nk:nk + 1], pwn[0:1, ts(nk, 128)], idf1[:]
                    )
                pwc = pgel.tile([128, 4], BF16, name="pwc", tag="pwc")
                nc.scalar.activation(pwc[:], ppw[:], AF.Copy)
                # pooled row (LN_f affine deferred into Wf1'/bf1' host folds)
                prow_sb = pgel.tile([1, D], BF16, name="prow", tag="prow")
                for dh in range(2):
                    prow = ps_small.tile([1, 512], F32, name="prow_ps", tag="sm")
                    for nk in range(4):
                        nc.tensor.matmul(
                            prow[:], pwc[:, nk:nk + 1],
                            fN[:, nk, dh * 512:(dh + 1) * 512],
                            start=(nk == 0), stop=(nk == 3),
                        )
                    nc.vector.tensor_copy(
                        prow_sb[0:1, dh * 512:(dh + 1) * 512], prow[:]
                    )
                nc.sync.dma_start(pool16[i:i + 1, :], prow_sb[0:1, :])

            for i in range(NB + 3):
                if i < NB:
                    p1_emb(i)
                if 1 <= i <= NB:
                    p2_big(i - 1)
                if 3 <= i:
                    p4a(i - 3)
                if 1 <= i <= NB:
                    p2_fused(i - 1)
                if 3 <= i:
                    p4b(i - 3)
                if i < NB:
                    p1_stats(i)
                    p1_tr(i)
                if 2 <= i <= NB + 1:
                    p3(i - 2)

            # =================== fc head ==============================
            # Wf2 @ Wf3 folded host-side (no nonlinearity between them):
            # rul = |lrelu(pooled @ Wf1' + bf1') @ W23 + b23|
            wts1 = []
            for t in range(32):
                wt = pw3.tile([128, 512], HDT, name="wt1", tag="w3")
                nc.sync.dma_start(wt[:], WF1[t])
                wts1.append(wt)
            # pooledT [d-part, batch] from pool16 [batch, d]
            for dk in range(8):
                ptp = ps_small.tile([128, NB], BF16, name="ptp", tag="sm")
                nc.tensor.transpose(
                    ptp[:], pool16[0:NB, ts(dk, 128)], id_sb[0:NB, 0:NB]
                )
                nc.scalar.activation(pooledT[:, dk, :], ptp[:], AF.Copy)
            # h1 = lrelu(pooled @ Wf1' + bf1')  [batch-part 16, DF]
            h1 = pgel.tile([NB, DF], BF16, name="h1", tag="h1")
            for fg in range(4):
                hg = ps_main.tile([NB, 512], F32, name="hg", tag="pm")
                for kd in range(8):
                    nc.tensor.matmul(
                        hg[:], pooledT[:, kd, :], wts1[fg * 8 + kd][:],
                        start=(kd == 0), stop=False,
                    )
                nc.tensor.matmul(
                    hg[:], ones1[0:1, :], bf1r_sb[0:1, ts(fg, 512)],
                    start=False, stop=True,
                )
                nc.scalar.activation(
                    h1[0:NB, ts(fg, 512)], hg[:], AF_LRELU, alpha=0.01
                )
            # rul = |h1 . W23 + b23| as a per-partition DVE dot (batch-part)
            rsum4 = psc2.tile([NB, 4], F32, name="rsum4", tag="rsum4")
            for fg in range(4):
                hm = pgel.tile([NB, 512], BF16, name="hm", tag="hm")
                nc.vector.tensor_tensor(
                    hm[:], h1[0:NB, ts(fg, 512)], w23bc_sb[0:NB, ts(fg, 512)],
                    op=ALU.mult,
                )
                nc.vector.tensor_reduce(
                    rsum4[:, fg:fg + 1], hm[:], axis=AX.X, op=ALU.add
                )
            rsum = psc2.tile([NB, 1], F32, name="rsum", tag="rsum")
            nc.vector.tensor_reduce(rsum[:], rsum4[:], axis=AX.X, op=ALU.add)
            bf3_t = psc2.tile([NB, 1], F32, name="bf3_t", tag="bf3")
            nc.gpsimd.memset(bf3_t[:], b23_f)
            rul_sb = psc2.tile([NB, 1], F32, name="rul_sb", tag="rul")
            nc.scalar.activation(rul_sb[:], rsum[:], AF.Abs, bias=bf3_t[:])
            nc.sync.dma_start(RUL[:], rul_sb[:])

        wts.release()
        glob.release()

    nc.compile()
    return nc


def _prep_in_maps(inputs):
    f32 = np.float32
    x_enc = np.asarray(inputs["x_enc"], f32)
    W_emb = np.asarray(inputs["W_emb"], f32)
    b_emb = np.asarray(inputs["b_emb"], f32)
    g_s = np.asarray(inputs["g_s"], f32)
    b_s = np.asarray(inputs["b_s"], f32)
    basis = np.asarray(inputs["basis"], np.float64)
    Wq = np.asarray(inputs["Wq"], np.float64)
    bq = np.asarray(inputs["bq"], f32)
    Wk = np.asarray(inputs["Wk"], np.float64)
    bk = np.asarray(inputs["bk"], f32)
    Wv = np.asarray(inputs["Wv"], f32)
    bv = np.asarray(inputs["bv"], f32)
    g_f = np.asarray(inputs["g_f"], f32)
    b_f = np.asarray(inputs["b_f"], f32)
    Wp1 = np.asarray(inputs["Wp1"], f32)
    bp1 = np.asarray(inputs["bp1"], f32)
    Wp2 = np.asarray(inputs["Wp2"], f32)
    Wf1 = np.asarray(inputs["Wf1"], np.float64)
    bf1 = np.asarray(inputs["bf1"], np.float64)
    Wf2 = np.asarray(inputs["Wf2"], np.float64)
    bf2 = np.asarray(inputs["bf2"], np.float64)
    Wf3 = np.asarray(inputs["Wf3"], np.float64)
    b_f64 = np.asarray(inputs["b_f"], np.float64)
    g_f64 = np.asarray(inputs["g_f"], np.float64)

    assert not (np.any(bq) or np.any(bk)), "folded QK path requires bq=bk=0"

    M = (Wq @ Wk.T).astype(f32)
    Wqb = (Wq @ basis.T).astype(f32)

    wp1g = np.zeros((D, 128), f32)
    wp1g[:, :H] = g_f[:, None] * Wp1
    bp1e = (b_f @ Wp1 + bp1).reshape(H, 1).astype(f32)

    # head folds: LN_f affine into Wf1/bf1; Wf2@Wf3 into one vector
    Wf1g = (g_f64[:, None] * Wf1).astype(f32)
    bf1g = (bf1 + b_f64 @ Wf1).astype(f32)
    W23 = (Wf2 @ Wf3[:, 0]).astype(f32)
    b23 = float(bf2 @ Wf3[:, 0])

    e_dt = "e4m3" if EMB_FP8 else "bf16"
    m_dt = "e4m3" if ATTN_FP8 else "bf16"
    v_dt = "e4m3" if V_FP8 else "bf16"
    h_dt = "bf16" if HEAD_DT == "bf16" else "f32r"

    common = {
        "wemb": _cast(_sbuf_layout(W_emb * S_EMB, 8), e_dt),
        "m_mat": _cast(_sbuf_layout(M * S_M, 8), m_dt),
        "wqb": _cast(_sbuf_layout(Wqb * S_WQB, 8), m_dt),
        "wv": _cast(_sbuf_layout(Wv * S_WV, 8), v_dt),
        "wp1g": _cast(_sbuf_layout(wp1g, 8), "bf16"),
        "wp2": Wp2.astype(BFNP),
        "wf1": _cast(
            Wf1g.reshape(8, 128, 4, 512).transpose(2, 0, 1, 3).reshape(32, 128, 512),
            h_dt,
        ),
        "w23bc": _cast(np.tile(W23.reshape(1, DF), (NB, 1)), "bf16"),
        "gs_c": _chunk_major(g_s, 8),
        "bs_c": _chunk_major(b_s, 8),
        "bp1e": bp1e,
        "bf1r": _cast(bf1g.reshape(1, DF), "bf16"),
        "ident": np.eye(128).astype(BFNP),
    }
    bemb_nz = bool(np.any(b_emb))
    bv_nz = bool(np.any(bv))
    if bemb_nz:
        common["bemb_row"] = _round_f32r(b_emb.reshape(1, D) * S_EMB)
    if bv_nz:
        common["bv_row"] = _round_f32r(bv.reshape(1, D) * S_WV)

    in_maps = []
    for c in range(NCORES):
        m = dict(common)
        xs = x_enc[c * NB:(c + 1) * NB]
        m["x"] = _cast(
            xs.reshape(NB, 8, 128, C).transpose(0, 2, 1, 3), e_dt
        )
        in_maps.append(m)

    age_scale_f = float(np.asarray(inputs["age_scale"], f32))
    bf3_f = float(np.asarray(inputs["bf3"], np.float64).reshape(-1)[0])
    b23_f = b23 + bf3_f
    s_triv = bool(np.allclose(g_s, 1.0) and not np.any(b_s))
    return in_maps, age_scale_f, b23_f, bemb_nz, bv_nz, s_triv


_NC_CACHE = {}


def build_program(inputs, sim_acts=False):
    in_maps, age_scale_f, b23_f, bemb_nz, bv_nz, s_triv = _prep_in_maps(inputs)
    key = (age_scale_f, b23_f, bemb_nz, bv_nz, s_triv, sim_acts)
    if key not in _NC_CACHE:
        _NC_CACHE[key] = _build(
            age_scale_f, b23_f, bemb_nz, bv_nz, s_triv, sim_acts
        )
    return _NC_CACHE[key], in_maps


def kernel(**inputs):
    from concourse.bass_utils import run_bass_kernel_spmd

    nc, in_maps = build_program(inputs)
    res = run_bass_kernel_spmd(nc, in_maps, core_ids=list(range(NCORES)))
    out = np.concatenate(
        [res.results[c]["rul"] for c in range(NCORES)], axis=0
    ).astype(np.float32)
    return out

